# revision 10
# baseline (speedup 1.0000x reference)
"""Trainium2 Bass kernel for nn_Decoder_34325378630277 (FNO-UNet decoder).

Sharding: 8 cores = 2 batches x 4 row-quarters (64 owned rows each).
Conv halos handled by extended recompute (host supplies zero-padded row
slices). FNO spectral path: only 4x4 low modes survive, so the forward
projection is per-channel matmuls against 32 cos/sin basis components
(pixel-major slabs obtained via bf16 DMA transpose through DRAM), the
channel mix is applied to per-core partials (mix commutes with the row
sum), the tiny mixed coefficients are AllReduced, and the inverse
expansion is accumulated directly into the 1x1-conv PSUM. Convs run fp32
(PE rate is dtype-independent); the spectral path runs bf16 (validated
~1e-6 output impact).
"""
import sys, types

sys.path.insert(0, "/opt/trn_rl_repo")
import numpy as np
import ml_dtypes

# NTFF profile hook shim (lets trace=True work under axon; harmless otherwise)
try:
    import antenv  # noqa: F401
    if "antenv.axon_hooks" not in sys.modules:
        _h = {"hook": None}
        _m = types.ModuleType("antenv.axon_hooks")
        _m.set_axon_ntff_profile_hook = lambda h: _h.__setitem__("hook", h)
        _m.get_axon_ntff_profile_hook = lambda: _h["hook"]
        sys.modules["antenv.axon_hooks"] = _m
        from trn_agent_boot.trn_boot import _ntff_profile_via_ctypes
        _m.set_axon_ntff_profile_hook(
            _ntff_profile_via_ctypes("/opt/axon/libaxon_pjrt.so"))
except Exception:
    pass

import concourse.bass as bass
import concourse.bacc as bacc
import concourse.tile as tile
from concourse import mybir, masks
from concourse.bass_utils import run_bass_kernel_spmd

F32 = mybir.dt.float32
BF16 = mybir.dt.bfloat16
AF = mybir.ActivationFunctionType

B, HH, WW, NF = 2, 256, 256, 16
OWN = 64
NCORES = 8
WP = WW + 2          # padded width (zero cols at 0 and WP-1)
EMAX = 9             # x5u halo
M = 4                # modes kept per axis

# fno stages: (tag, Ci, Co, e_dst)
FNOS = [("f5", 256, 128, 9), ("f6", 128, 64, 6), ("f7", 64, 32, 3), ("f8", 32, 16, 1)]
# conv blocks: (tag, C, n_stages, e_dst per stage)
CONVS = [("c6", 128, 3, [8, 7, 6]), ("c7", 64, 3, [5, 4, 3]),
         ("c8", 32, 2, [2, 1]), ("c9", 16, 1, [0])]


def _rows(e):
    return OWN + 2 * e


# ---------------------------------------------------------------------------
# device program
# ---------------------------------------------------------------------------

def _build_nc():
    nc = bacc.Bacc("TRN2", target_bir_lowering=False, debug=False,
                   num_devices=NCORES)

    def din(name, shape, dt):
        return nc.dram_tensor(name, list(shape), dt, kind="ExternalInput").ap()

    # --- external inputs (per-core data) ---
    x5_sl = din("x5_sl", (256, _rows(9) * WW), F32)
    x4_sl = din("x4_sl", (128, _rows(9) * WW), F32)
    x3_sl = din("x3_sl", (64, _rows(6) * WW), F32)
    x2_sl = din("x2_sl", (32, _rows(3) * WW), F32)
    x1_sl = din("x1_sl", (16, _rows(1) * WW), F32)
    skips = {"f5": x4_sl, "f6": x3_sl, "f7": x2_sl, "f8": x1_sl}
    x5T = din("x5T", (2, OWN, 128, 256), BF16)          # (wt, h, w, c)
    fb_in = din("fb", (128, 2 * OWN * 32), BF16)        # (w, (wt h m)) fwd basis
    gb = din("gb", (32, _rows(9) * WW), BF16)           # inv basis rows r0-9..r0+73
    mask_r = din("mask_r", (2, _rows(9)), F32)          # [in-image, 1-in-image]
    bsel = din("bsel", (32, 2), F32)                    # one-hot batch select
    cw = {}
    cb = {}
    for tag, C, nst, _ in CONVS:
        cw[tag] = din(tag + "w", (nst, 9, C, C), F32)
        cb[tag] = din(tag + "b", (2, nst * C), F32)
    bwT = {}
    bb = {}
    mixw = {}
    for tag, Ci, Co, _ in FNOS:
        kt = (Ci + 127) // 128
        bwT[tag] = din(tag + "_bwT", (128, kt * Co), F32)
        bb[tag] = din(tag + "_bb", (2, Co), F32)
        mixw[tag] = din(tag + "_mixw", (16, 2, Ci, Co), BF16)
    owT = din("owT", (16, 2), F32)
    ob = din("ob", (2, 1), F32)

    out_sl = nc.dram_tensor("out_sl", [2, OWN * WW], F32,
                            kind="ExternalOutput").ap()

    # --- internal dram ---
    xb_d = {}
    for tag, Ci, Co, e in FNOS[1:]:
        xb_d[tag] = nc.dram_tensor("xb_" + tag, [OWN, Ci, 256], BF16).ap()
    cc_in = {}
    cc_out = {}
    for tag, Ci, Co, e in FNOS:
        cc_in[tag] = nc.dram_tensor("ccin_" + tag, [2, 32, Co], F32).ap()
        cc_out[tag] = nc.dram_tensor("ccout_" + tag, [2, 32, Co], F32,
                                     addr_space="Shared").ap()
    ccw_in = nc.dram_tensor("ccw_in", [1, 8], F32).ap()
    ccw_out = nc.dram_tensor("ccw_out", [1, 8], F32, addr_space="Shared").ap()
    ALL8 = [list(range(NCORES))]

    with tile.TileContext(nc) as tc:
        import contextlib
        est = contextlib.ExitStack()
        with est:
            big = est.enter_context(tc.tile_pool(name="big", bufs=1))
            cst = est.enter_context(tc.tile_pool(name="cst", bufs=1))
            stm2 = est.enter_context(tc.tile_pool(name="stm2", bufs=2))
            stm3 = est.enter_context(tc.tile_pool(name="stm3", bufs=3))
            tmp1 = est.enter_context(tc.tile_pool(name="tmp1", bufs=1))
            ppool = est.enter_context(
                tc.tile_pool(name="ppool", bufs=3, space="PSUM"))
            pspec = est.enter_context(
                tc.tile_pool(name="pspec", bufs=1, space="PSUM"))

            # warmup collective (absorbs communicator init early)
            nc.gpsimd.collective_compute(
                "AllReduce", mybir.AluOpType.add, replica_groups=ALL8,
                ins=[ccw_in[:]], outs=[ccw_out[:]])

            # --- constants resident in SBUF ---
            ident = cst.tile([128, 128], BF16, tag="ident")
            masks.make_identity(nc, ident[:])
            mask_t = cst.tile([2, _rows(9)], F32, tag="mask")
            nc.sync.dma_start(mask_t[:], mask_r[:])
            bsel_t = cst.tile([32, 2], F32, tag="bsel")
            nc.sync.dma_start(bsel_t[:], bsel[:])
            fb_t = cst.tile([128, 2 * OWN * 32], BF16, tag="fb")
            nc.sync.dma_start(fb_t[:], fb_in[:])

            def fb_sl(wt, h):  # (128, 32) lhsT slab
                o = (wt * OWN + h) * 32
                return fb_t[:, o:o + 32]

            bwT_t = {}
            bb_t = {}
            for tag, Ci, Co, e in FNOS:
                kt = (Ci + 127) // 128
                bwT_t[tag] = cst.tile([128, kt * Co], F32, tag="bw" + tag, name="bw_" + tag)
                nc.sync.dma_start(bwT_t[tag][:], bwT[tag][:])
                bb_t[tag] = cst.tile([2, Co], F32, tag="bb" + tag, name="bbt_" + tag)
                nc.sync.dma_start(bb_t[tag][:], bb[tag][:])
            cb_t = {}
            for tag, C, nst, _ in CONVS:
                cb_t[tag] = cst.tile([2, nst * C], F32, tag="cb" + tag, name="cbt_" + tag)
                nc.sync.dma_start(cb_t[tag][:], cb[tag][:])
            ow_t = cst.tile([16, 2], F32, tag="ow")
            nc.sync.dma_start(ow_t[:], owT[:])
            ob_t = cst.tile([2, 1], F32, tag="ob")
            nc.sync.dma_start(ob_t[:], ob[:])

            # --- big activation slots ---
            def new_act(slot, C, e):
                t = big.tile([128, _rows(e) * WP], F32, tag=slot)
                z = t[:C].rearrange("c (r w) -> c r w", w=WP)
                nc.vector.memset(z[:, :, 0:1], 0.0)
                nc.vector.memset(z[:, :, WP - 1:WP], 0.0)
                return t

            def act_view(t, C, e):
                return t[:C].rearrange("c (r w) -> c r w", w=WP)

            def mask_rhs(rd, moff, nr):
                return (mask_t[:, rd + moff:rd + moff + nr]
                        .broadcast_to((2, nr, WW)))

            # ---------------- fno block ----------------
            def fno_block(tag, Ci, Co, e, src_t, src_e, dst_t):
                kt = (Ci + 127) // 128
                # ---- forward projection -> partial xf (32, Ci) ----
                pxf = pspec.tile([32, Ci], F32, tag="pxf")
                n_mm = 2 * OWN
                mm_i = 0
                if tag == "f5":
                    HB = 4
                    for wt in range(2):
                        for hb in range(OWN // HB):
                            ch = stm3.tile([128, HB * 256], BF16, tag="xtc")
                            nc.sync.dma_start(
                                ch[:].rearrange("w (h c) -> w h c", c=256),
                                x5T[wt, hb * HB:(hb + 1) * HB]
                                .rearrange("h w c -> w h c"))
                            for h in range(HB):
                                nc.tensor.matmul(
                                    pxf[:], fb_sl(wt, hb * HB + h),
                                    ch[:, h * 256:(h + 1) * 256],
                                    start=(mm_i == 0), stop=(mm_i == n_mm - 1))
                                mm_i += 1
                else:
                    # cast owned rows to bf16, bounce through DRAM, read back
                    # transposed as (w, (h c)) slabs
                    HBC = 4
                    sv = act_view(src_t, Ci, src_e)
                    for hb in range(OWN // HBC):
                        cc2 = stm3.tile([128, HBC * 256], BF16, tag="xtc", name="castch")
                        nc.vector.tensor_copy(
                            cc2[:Ci].rearrange("c (h w) -> c h w", w=256),
                            sv[:, src_e + hb * HBC:src_e + (hb + 1) * HBC,
                               1:WP - 1])
                        nc.sync.dma_start(
                            xb_d[tag][hb * HBC:(hb + 1) * HBC]
                            .rearrange("h c w -> c h w"),
                            cc2[:Ci].rearrange("c (h w) -> c h w", w=256))
                    xb_flat = xb_d[tag][:].rearrange("h c w -> (h c) w")
                    HB = 8
                    for wt in range(2):
                        for hb in range(OWN // HB):
                            xt = stm3.tile([128, HB * Ci], BF16, tag="xtc")
                            nc.sync.dma_start_transpose(
                                out=xt[:],
                                in_=xb_flat[hb * HB * Ci:(hb + 1) * HB * Ci,
                                            wt * 128:(wt + 1) * 128])
                            for h in range(HB):
                                nc.tensor.matmul(
                                    pxf[:], fb_sl(wt, hb * HB + h),
                                    xt[:, h * Ci:(h + 1) * Ci],
                                    start=(mm_i == 0), stop=(mm_i == n_mm - 1))
                                mm_i += 1
                xf = tmp1.tile([32, Ci], BF16, tag="xf")
                nc.vector.tensor_copy(xf[:], pxf[:])

                # ---- transpose xf -> xfT (Ci, 32); swapped/negated copy ----
                xfT = tmp1.tile([128, kt * 32], BF16, tag="xfT")
                for k in range(kt):
                    pt = pspec.tile([128, 32], BF16, tag="ptr")
                    w = min(128, Ci - k * 128)
                    nc.tensor.transpose(pt[:w, :], xf[:, k * 128:k * 128 + w],
                                        ident[:32, :32])
                    nc.vector.tensor_copy(xfT[:w, k * 32:(k + 1) * 32],
                                          pt[:w, :])
                xfN = tmp1.tile([128, kt * 32], BF16, tag="xfN")
                xfT3 = xfT[:].rearrange("p (a b) -> p a b", b=2)
                xfN3 = xfN[:].rearrange("p (a b) -> p a b", b=2)
                for k in range(kt):
                    mlo = k * 16
                    nc.scalar.mul(xfN3[:, mlo:mlo + 16, 0:1],
                                  xfT3[:, mlo:mlo + 16, 1:2], -1.0)
                    nc.vector.tensor_copy(xfN3[:, mlo:mlo + 16, 1:2],
                                          xfT3[:, mlo:mlo + 16, 0:1])

                # ---- mix (per mode) -> of (Co, 32) ----
                pof = pspec.tile([Co, 32], F32, tag="pof")
                for mu in range(16):
                    n_grp = 2 * kt
                    gi = 0
                    for k in range(kt):
                        w = min(128, Ci - k * 128)
                        mws = stm3.tile([128, 2 * Co], BF16, tag="mixw")
                        nc.sync.dma_start(
                            mws[:w].rearrange("i (j o) -> i j o", j=2),
                            mixw[tag][mu, :, k * 128:k * 128 + w]
                            .rearrange("j i o -> i j o"))
                        nc.tensor.matmul(
                            pof[:, 2 * mu:2 * mu + 2], mws[:w, :Co],
                            xfT[:w, k * 32 + 2 * mu:k * 32 + 2 * mu + 2],
                            start=(gi == 0), stop=(gi == n_grp - 1))
                        gi += 1
                        nc.tensor.matmul(
                            pof[:, 2 * mu:2 * mu + 2], mws[:w, Co:2 * Co],
                            xfN[:w, k * 32 + 2 * mu:k * 32 + 2 * mu + 2],
                            start=(gi == 0), stop=(gi == n_grp - 1))
                        gi += 1
                of_sb = tmp1.tile([Co, 32], BF16, tag="of_sb")
                nc.vector.tensor_copy(of_sb[:], pof[:])

                # ---- transpose of -> (32, Co); batch-select; AllReduce ----
                pofT = pspec.tile([32, 128], BF16, tag="ptr")
                nc.tensor.transpose(pofT[:, :Co], of_sb[:], ident[:Co, :Co])
                s0 = tmp1.tile([32, Co], F32, tag="sa", name="s0")
                s1 = tmp1.tile([32, Co], F32, tag="sb", name="s1")
                nc.scalar.activation(s0[:], pofT[:, :Co], AF.Copy,
                                     scale=bsel_t[:, 0:1])
                nc.scalar.activation(s1[:], pofT[:, :Co], AF.Copy,
                                     scale=bsel_t[:, 1:2])
                nc.sync.dma_start(cc_in[tag][0], s0[:])
                nc.sync.dma_start(cc_in[tag][1], s1[:])
                nc.gpsimd.collective_compute(
                    "AllReduce", mybir.AluOpType.add, replica_groups=ALL8,
                    ins=[cc_in[tag][:]], outs=[cc_out[tag][:]])
                t0 = tmp1.tile([32, Co], F32, tag="sa", name="t0")
                t1 = tmp1.tile([32, Co], F32, tag="sb", name="t1")
                nc.sync.dma_start(t0[:], cc_out[tag][0])
                nc.sync.dma_start(t1[:], cc_out[tag][1])
                u0 = tmp1.tile([32, Co], F32, tag="u0")
                nc.scalar.activation(u0[:], t0[:], AF.Copy,
                                     scale=bsel_t[:, 0:1])
                u1 = tmp1.tile([32, Co], F32, tag="u1")
                nc.scalar.activation(u1[:], t1[:], AF.Copy,
                                     scale=bsel_t[:, 1:2])
                nc.vector.tensor_add(u0[:], u0[:], u1[:])
                ofb = tmp1.tile([32, Co], BF16, tag="ofb")
                nc.vector.tensor_copy(ofb[:], u0[:])

                # ---- inverse expansion + 1x1 conv + gelu + skip ----
                moff = EMAX - e
                dv = act_view(dst_t, Co, e)
                nr = 1 if tag == "f5" else 2       # rows per chunk
                npix = nr * WW
                for c_i in range(_rows(e) // nr):
                    rd = nr * c_i
                    ps = ppool.tile([Co, npix], F32, tag="cpsum")
                    if tag == "f5":
                        xch = stm2.tile([128, kt * npix], F32, tag="cwx")
                        for k in range(kt):
                            nc.sync.dma_start(
                                xch[:, k * npix:(k + 1) * npix],
                                x5_sl[k * 128:(k + 1) * 128,
                                      rd * WW:(rd + nr) * WW])
                        for k in range(kt):
                            nc.tensor.matmul(
                                ps[:], bwT_t[tag][:, k * Co:(k + 1) * Co],
                                xch[:, k * npix:(k + 1) * npix],
                                start=(k == 0), stop=False)
                    else:
                        sv = act_view(src_t, Ci, src_e)
                        nc.tensor.matmul(
                            ps[:], bwT_t[tag][:Ci, :Co],
                            sv[:, rd:rd + nr, 1:WP - 1],
                            start=True, stop=False)
                    gch = stm2.tile([32, npix], BF16, tag="gbch")
                    nc.sync.dma_start(
                        gch[:], gb[:, (rd + moff) * WW:(rd + moff + nr) * WW])
                    nc.tensor.matmul(ps[:], ofb[:, :Co], gch[:],
                                     start=False, stop=False)
                    nc.tensor.matmul(ps[:], bb_t[tag][:, :Co],
                                     mask_rhs(rd, moff, nr),
                                     start=False, stop=True)
                    dslice = dv[:, rd:rd + nr, 1:WP - 1]
                    nc.scalar.activation(
                        dslice, ps[:].rearrange("c (a w) -> c a w", w=WW),
                        AF.Gelu)
                    sk = stm2.tile([Co, npix], F32, tag="skipch")
                    nc.sync.dma_start(sk[:],
                                      skips[tag][:Co, rd * WW:(rd + nr) * WW])
                    nc.vector.tensor_add(
                        dslice, dslice,
                        sk[:].rearrange("c (a w) -> c a w", w=WW))

            # ---------------- conv stage ----------------
            def conv_stage(tag, C, st, e, src_t, src_e, dst_t):
                moff = EMAX - e
                wsl = stm2.tile([128, 9 * C], F32, tag="cwx")
                nc.sync.dma_start(wsl[:C].rearrange("i (t o) -> i t o", t=9),
                                  cw[tag][st].rearrange("t i o -> i t o"))
                sv = act_view(src_t, C, src_e)
                dvv = act_view(dst_t, C, e)
                for c_i in range(_rows(e) // 2):
                    rd = 2 * c_i
                    ps = ppool.tile([C, 512], F32, tag="cpsum")
                    for t9 in range(9):
                        dy, dx = t9 // 3 - 1, t9 % 3 - 1
                        nc.tensor.matmul(
                            ps[:], wsl[:C, t9 * C:(t9 + 1) * C],
                            sv[:, rd + 1 + dy:rd + 3 + dy,
                               1 + dx:WP - 1 + dx],
                            start=(t9 == 0), stop=False)
                    nc.tensor.matmul(ps[:], cb_t[tag][:, st * C:st * C + C],
                                     mask_rhs(rd, moff, 2),
                                     start=False, stop=True)
                    nc.scalar.activation(
                        dvv[:, rd:rd + 2, 1:WP - 1],
                        ps[:].rearrange("c (a w) -> c a w", w=WW), AF.Relu)

            # ---------------- the network ----------------
            x5u = new_act("A", 128, 9)
            fno_block("f5", 256, 128, 9, None, None, x5u)
            x6a = new_act("B", 128, 8)
            conv_stage("c6", 128, 0, 8, x5u, 9, x6a)
            x6b = new_act("A", 128, 7)
            conv_stage("c6", 128, 1, 7, x6a, 8, x6b)
            x6 = new_act("B", 128, 6)
            conv_stage("c6", 128, 2, 6, x6b, 7, x6)
            x6u = new_act("A", 64, 6)
            fno_block("f6", 128, 64, 6, x6, 6, x6u)
            x7a = new_act("B", 64, 5)
            conv_stage("c7", 64, 0, 5, x6u, 6, x7a)
            x7b = new_act("A", 64, 4)
            conv_stage("c7", 64, 1, 4, x7a, 5, x7b)
            x7 = new_act("B", 64, 3)
            conv_stage("c7", 64, 2, 3, x7b, 4, x7)
            x7u = new_act("A", 32, 3)
            fno_block("f7", 64, 32, 3, x7, 3, x7u)
            x8a = new_act("B", 32, 2)
            conv_stage("c8", 32, 0, 2, x7u, 3, x8a)
            x8 = new_act("A", 32, 1)
            conv_stage("c8", 32, 1, 1, x8a, 2, x8)
            x8u = new_act("B", 16, 1)
            fno_block("f8", 32, 16, 1, x8, 1, x8u)
            x9 = new_act("A", 16, 0)
            conv_stage("c9", 16, 0, 0, x8u, 1, x9)

            # final 1x1 conv (16 -> 2), owned rows only
            x9v = act_view(x9, 16, 0)
            for c_i in range(OWN // 2):
                rd = 2 * c_i
                ps = ppool.tile([2, 512], F32, tag="cpsum")
                nc.tensor.matmul(ps[:], ow_t[:], x9v[:, rd:rd + 2, 1:WP - 1],
                                 start=True, stop=True)
                oc = stm2.tile([2, 512], F32, tag="skipch", name="outch")
                nc.scalar.activation(oc[:], ps[:], AF.Identity, bias=ob_t[:])
                nc.sync.dma_start(out_sl[:, rd * WW:(rd + 2) * WW], oc[:])

    nc.compile()
    return nc


# ---------------------------------------------------------------------------
# host side
# ---------------------------------------------------------------------------

def _slice_rows(x, lo, hi):
    """x: (C, 256, 256) -> (C, hi-lo, 256) zero-padded out of range."""
    C = x.shape[0]
    out = np.zeros((C, hi - lo, WW), np.float32)
    a, b2 = max(lo, 0), min(hi, HH)
    if b2 > a:
        out[:, a - lo:b2 - lo] = x[:, a:b2]
    return out


def _host_inputs(inputs):
    i = inputs
    maps = []
    kk, ll = np.meshgrid(np.arange(M), np.arange(M), indexing="ij")
    kf = kk.reshape(-1).astype(np.float64)   # mu = 4k + l
    lf = ll.reshape(-1).astype(np.float64)
    alpha32 = np.where(lf == 0, 1.0, 2.0).repeat(2)  # per 32-comp row

    def basis(rows_abs, wvals):  # -> (32, len(rows), len(w))
        th = 2 * np.pi * (kf[:, None, None] * rows_abs[None, :, None] / HH
                          + lf[:, None, None] * wvals[None, None, :] / WW)
        out = np.empty((32, len(rows_abs), len(wvals)), np.float32)
        out[0::2] = np.cos(th) / 256.0
        out[1::2] = -np.sin(th) / 256.0
        return out

    # weights (identical on all cores)
    cw_np = {}
    cb_np = {}
    for tag, C, nst, _ in CONVS:
        w = np.asarray(i[tag + "_w"], np.float32)     # (n, co, ci, 3, 3)
        cw_np[tag] = np.ascontiguousarray(
            w.transpose(0, 3, 4, 2, 1).reshape(nst, 9, C, C))
        cb_np[tag] = np.stack([
            np.asarray(i[tag + "_b"], np.float32).reshape(nst * C),
            np.full(nst * C, -1e4, np.float32)])
    fno_np = {}
    for tag, Ci, Co, e in FNOS:
        kt = (Ci + 127) // 128
        bw = np.asarray(i[tag + "_bw"], np.float32)[:, :, 0, 0]  # (oc, ic)
        full = np.ascontiguousarray(bw.T)                        # (ic, oc)
        bwT_ = np.zeros((128, kt * Co), np.float32)
        for k in range(kt):
            w = min(128, Ci - k * 128)
            bwT_[:w, k * Co:(k + 1) * Co] = full[k * 128:k * 128 + w]
        wr = np.asarray(i[tag + "_wr"], np.float32)   # (ic, oc, 4, 4)
        wi = np.asarray(i[tag + "_wi"], np.float32)
        mw = np.zeros((16, 2, Ci, Co), np.float32)
        for mu in range(16):
            k, l = mu // 4, mu % 4
            mw[mu, 0] = wr[:, :, k, l]
            mw[mu, 1] = wi[:, :, k, l]
        fno_np[tag] = (bwT_,
                       np.stack([np.asarray(i[tag + "_bb"], np.float32),
                                 np.full(Co, -1e4, np.float32)]),
                       mw.astype(ml_dtypes.bfloat16))
    owT_np = np.ascontiguousarray(
        np.asarray(i["ow"], np.float32)[:, :, 0, 0].T)           # (16, 2)
    ob_np = np.asarray(i["ob"], np.float32)[:, None]

    skips_full = {"f5": np.asarray(i["x4"], np.float32),
                  "f6": np.asarray(i["x3"], np.float32),
                  "f7": np.asarray(i["x2"], np.float32),
                  "f8": np.asarray(i["x1"], np.float32)}
    x5 = np.asarray(i["x5"], np.float32)
    wvals = np.arange(WW, dtype=np.float64)

    for core in range(NCORES):
        b, q = divmod(core, 4)
        r0 = OWN * q
        m = {}
        m["x5_sl"] = _slice_rows(x5[b], r0 - 9, r0 + OWN + 9).reshape(256, -1)
        m["x4_sl"] = _slice_rows(skips_full["f5"][b], r0 - 9, r0 + OWN + 9).reshape(128, -1)
        m["x3_sl"] = _slice_rows(skips_full["f6"][b], r0 - 6, r0 + OWN + 6).reshape(64, -1)
        m["x2_sl"] = _slice_rows(skips_full["f7"][b], r0 - 3, r0 + OWN + 3).reshape(32, -1)
        m["x1_sl"] = _slice_rows(skips_full["f8"][b], r0 - 1, r0 + OWN + 1).reshape(16, -1)
        xo = x5[b][:, r0:r0 + OWN, :]                       # (256c, 64h, 256w)
        x5T_ = xo.transpose(1, 2, 0).reshape(OWN, 2, 128, 256).transpose(1, 0, 2, 3)
        m["x5T"] = np.ascontiguousarray(x5T_).astype(ml_dtypes.bfloat16)
        # fwd basis (w, (wt h m)) at abs rows r0+h, col wt*128+w
        fbb = basis(np.arange(r0, r0 + OWN, dtype=np.float64), wvals)  # (32,64,256)
        fbb = (fbb.transpose(2, 1, 0)                       # (w256, h, m)
               .reshape(2, 128, OWN, 32)                    # (wt, w, h, m)
               .transpose(1, 0, 2, 3)                       # (w, wt, h, m)
               .reshape(128, 2 * OWN * 32))
        m["fb"] = np.ascontiguousarray(fbb).astype(ml_dtypes.bfloat16)
        rows = np.arange(r0 - 9, r0 + OWN + 9, dtype=np.float64)
        gbb = basis(rows, wvals) * alpha32[:, None, None]
        gbb[:, (rows < 0) | (rows >= HH), :] = 0.0
        m["gb"] = gbb.reshape(32, -1).astype(ml_dtypes.bfloat16)
        mrow = ((rows >= 0) & (rows < HH)).astype(np.float32)
        m["mask_r"] = np.stack([mrow, 1.0 - mrow])
        bs = np.zeros((32, 2), np.float32)
        bs[:, b] = 1.0
        m["bsel"] = bs
        for tag, C, nst, _ in CONVS:
            m[tag + "w"] = cw_np[tag]
            m[tag + "b"] = cb_np[tag]
        for tag, Ci, Co, e in FNOS:
            bwT_, bb_, mw_ = fno_np[tag]
            m[tag + "_bwT"] = bwT_
            m[tag + "_bb"] = bb_
            m[tag + "_mixw"] = mw_
        m["owT"] = owT_np
        m["ob"] = ob_np
        maps.append(m)
    return maps


_NC_CACHE = {}


def kernel(**inputs):
    if "nc" not in _NC_CACHE:
        _NC_CACHE["nc"] = _build_nc()
    nc = _NC_CACHE["nc"]
    maps = _host_inputs(inputs)
    res = run_bass_kernel_spmd(nc, maps, list(range(NCORES)), trace=False)
    out = np.zeros((B, 2, HH, WW), np.float32)
    for core in range(NCORES):
        b, q = divmod(core, 4)
        r0 = OWN * q
        out[b, :, r0:r0 + OWN, :] = res.results[core]["out_sl"].reshape(2, OWN, WW)
    return out


# revision 16
# speedup vs baseline: 3.0029x; 3.0029x over previous
"""Trainium2 Bass kernel for nn_Decoder_34325378630277 (FNO-UNet decoder).

Sharding: 8 cores = 2 batches x 4 row-quarters (64 owned rows each).
Conv halos handled by extended recompute (host supplies zero-padded row
slices). FNO spectral path: only 4x4 low modes survive, so the forward
projection is per-channel matmuls against 32 cos/sin basis components
(pixel-major slabs obtained via bf16 DMA transpose through DRAM), the
channel mix is applied to per-core partials (mix commutes with the row
sum), the tiny mixed coefficients are AllReduced, and the inverse
expansion is accumulated directly into the 1x1-conv PSUM. Convs run fp32
(PE rate is dtype-independent); the spectral path runs bf16 (validated
~1e-6 output impact).
"""
import sys, types

sys.path.insert(0, "/opt/trn_rl_repo")
import numpy as np
import ml_dtypes

# NTFF profile hook shim (lets trace=True work under axon; harmless otherwise)
try:
    import antenv  # noqa: F401
    if "antenv.axon_hooks" not in sys.modules:
        _h = {"hook": None}
        _m = types.ModuleType("antenv.axon_hooks")
        _m.set_axon_ntff_profile_hook = lambda h: _h.__setitem__("hook", h)
        _m.get_axon_ntff_profile_hook = lambda: _h["hook"]
        sys.modules["antenv.axon_hooks"] = _m
        from trn_agent_boot.trn_boot import _ntff_profile_via_ctypes
        _m.set_axon_ntff_profile_hook(
            _ntff_profile_via_ctypes("/opt/axon/libaxon_pjrt.so"))
except Exception:
    pass

import concourse.bass as bass
import concourse.bacc as bacc
import concourse.tile as tile
from concourse import mybir, masks
from concourse.bass_utils import run_bass_kernel_spmd

F32 = mybir.dt.float32
F32R = mybir.dt.float32r
BF16 = mybir.dt.bfloat16
AF = mybir.ActivationFunctionType

B, HH, WW, NF = 2, 256, 256, 16
OWN = 64
NCORES = 8
WP = WW + 2          # padded width (zero cols at 0 and WP-1)
EMAX = 9             # x5u halo
M = 4                # modes kept per axis

# fno stages: (tag, Ci, Co, e_dst)
FNOS = [("f5", 256, 128, 9), ("f6", 128, 64, 6), ("f7", 64, 32, 3), ("f8", 32, 16, 1)]
# conv blocks: (tag, C, n_stages, e_dst per stage)
CONVS = [("c6", 128, 3, [8, 7, 6]), ("c7", 64, 3, [5, 4, 3]),
         ("c8", 32, 2, [2, 1]), ("c9", 16, 1, [0])]


def _rows(e):
    return OWN + 2 * e


# ---------------------------------------------------------------------------
# device program
# ---------------------------------------------------------------------------

def _build_nc():
    nc = bacc.Bacc("TRN2", target_bir_lowering=False, debug=False,
                   num_devices=NCORES)

    def din(name, shape, dt):
        return nc.dram_tensor(name, list(shape), dt, kind="ExternalInput").ap()

    # --- external inputs (per-core data) ---
    x5_sl = din("x5_sl", (256, _rows(9) * WW), F32R)
    x4_sl = din("x4_sl", (128, _rows(9) * WW), F32)
    x3_sl = din("x3_sl", (64, _rows(6) * WW), F32)
    x2_sl = din("x2_sl", (32, _rows(3) * WW), F32)
    x1_sl = din("x1_sl", (16, _rows(1) * WW), F32)
    skips = {"f5": x4_sl, "f6": x3_sl, "f7": x2_sl, "f8": x1_sl}
    x5T = din("x5T", (2, OWN, 128, 256), BF16)          # (wt, h, w, c)
    zpad = din("zpad", (128, 2 * _rows(9)), F32R)       # zeros for pad cols
    fb_in = din("fb", (128, 2 * OWN * 32), BF16)        # (w, (wt h m)) fwd basis
    gb = din("gb", (32, _rows(9) * WW), BF16)           # inv basis rows r0-9..r0+73
    mask_pp = din("mask_pp", (128, _rows(9)), F32)      # in-image mask, replicated
    bsel = din("bsel", (32, 2), F32)                    # one-hot batch select
    cw = {}
    cb = {}
    for tag, C, nst, _ in CONVS:
        cw[tag] = din(tag + "w", (nst, 9, C, C), F32R)
        cb[tag] = din(tag + "b", (128, nst), F32)
    bwT = {}
    bb = {}
    mixw = {}
    for tag, Ci, Co, _ in FNOS:
        kt = (Ci + 127) // 128
        bwT[tag] = din(tag + "_bwT", (128, kt * Co), F32R)
        bb[tag] = din(tag + "_bb", (128, 1), F32)
        mixw[tag] = din(tag + "_mixw", (16, 2, Ci, Co), BF16)
    owT = din("owT", (16, 2), F32R)
    ob = din("ob", (2, 1), F32)

    out_sl = nc.dram_tensor("out_sl", [2, OWN * WW], F32,
                            kind="ExternalOutput").ap()

    # --- internal dram ---
    xb_d = {}
    for tag, Ci, Co, e in FNOS[1:]:
        xb_d[tag] = nc.dram_tensor("xb_" + tag, [OWN, Ci, 256], BF16).ap()
    cc_in = {}
    cc_out = {}
    for tag, Ci, Co, e in FNOS:
        cc_in[tag] = nc.dram_tensor("ccin_" + tag, [2, 32, Co], F32).ap()
        cc_out[tag] = nc.dram_tensor("ccout_" + tag, [2, 32, Co], F32,
                                     addr_space="Shared").ap()
    ccw_in = nc.dram_tensor("ccw_in", [1, 8], F32).ap()
    ccw_out = nc.dram_tensor("ccw_out", [1, 8], F32, addr_space="Shared").ap()
    ALL8 = [list(range(NCORES))]

    with tile.TileContext(nc) as tc:
        import contextlib
        est = contextlib.ExitStack()
        with est:
            big = est.enter_context(tc.tile_pool(name="big", bufs=1))
            cst = est.enter_context(tc.tile_pool(name="cst", bufs=1))
            stm2 = est.enter_context(tc.tile_pool(name="stm2", bufs=2))
            stm3 = est.enter_context(tc.tile_pool(name="stm3", bufs=3))
            tmp1 = est.enter_context(tc.tile_pool(name="tmp1", bufs=1))
            ppool = est.enter_context(
                tc.tile_pool(name="ppool", bufs=3, space="PSUM"))
            pspec = est.enter_context(
                tc.tile_pool(name="pspec", bufs=1, space="PSUM"))

            # warmup collective (absorbs communicator init early)
            nc.gpsimd.collective_compute(
                "AllReduce", mybir.AluOpType.add, replica_groups=ALL8,
                ins=[ccw_in[:]], outs=[ccw_out[:]])

            # --- constants resident in SBUF ---
            ident = cst.tile([128, 128], BF16, tag="ident")
            masks.make_identity(nc, ident[:])
            mask_t = cst.tile([128, _rows(9)], F32, tag="mask")
            nc.sync.dma_start(mask_t[:], mask_pp[:])
            bsel_t = cst.tile([32, 2], F32, tag="bsel")
            nc.sync.dma_start(bsel_t[:], bsel[:])
            fb_t = cst.tile([128, 2 * OWN * 32], BF16, tag="fb")
            nc.sync.dma_start(fb_t[:], fb_in[:])

            def fb_sl(wt, h):  # (128, 32) lhsT slab
                o = (wt * OWN + h) * 32
                return fb_t[:, o:o + 32]

            bwT_t = {}
            bb_t = {}
            for tag, Ci, Co, e in FNOS:
                kt = (Ci + 127) // 128
                bwT_t[tag] = cst.tile([128, kt * Co], F32R, tag="bw" + tag, name="bw_" + tag)
                nc.sync.dma_start(bwT_t[tag][:], bwT[tag][:])
                bb_t[tag] = cst.tile([128, 1], F32, tag="bb" + tag, name="bbt_" + tag)
                nc.sync.dma_start(bb_t[tag][:], bb[tag][:])
            cb_t = {}
            for tag, C, nst, _ in CONVS:
                cb_t[tag] = cst.tile([128, nst], F32, tag="cb" + tag, name="cbt_" + tag)
                nc.sync.dma_start(cb_t[tag][:], cb[tag][:])
            ow_t = cst.tile([16, 2], F32R, tag="ow")
            nc.sync.dma_start(ow_t[:], owT[:])
            ob_t = cst.tile([2, 1], F32, tag="ob")
            nc.sync.dma_start(ob_t[:], ob[:])

            # --- big activation slots ---
            def new_act(slot, C, e):
                t = big.tile([128, _rows(e) * WP], F32R, tag=slot)
                R = _rows(e)
                # zero the pad columns (cols 0 and WP-1 of every row)
                z = t[:].rearrange("c (r w) -> c r w", w=WP)
                nc.sync.dma_start(z[:, :, 0:1], zpad[:, :R])
                nc.sync.dma_start(z[:, :, WP - 1:WP], zpad[:, R:2 * R])
                return t

            def act_view(t, C, e):
                return t[:C].rearrange("c (r w) -> c r w", w=WP)

            def band_mask(dst_t, C, e):
                moff = EMAX - e
                R = _rows(e)
                dv = act_view(dst_t, C, e)
                for lo in (0, R - 9):
                    nc.vector.tensor_mul(
                        dv[:, lo:lo + 9, :], dv[:, lo:lo + 9, :],
                        mask_t[:C, moff + lo:moff + lo + 9]
                        .broadcast_to((C, 9, WP)))

            # ---------------- fno block ----------------
            def fno_block(tag, Ci, Co, e, src_t, src_e, dst_t):
                kt = (Ci + 127) // 128
                # ---- forward projection -> partial xf (32, Ci) ----
                pxf = pspec.tile([32, Ci], F32, tag="pxf")
                n_mm = 2 * OWN
                mm_i = 0
                if tag == "f5":
                    HB = 4
                    for wt in range(2):
                        for hb in range(OWN // HB):
                            ch = stm3.tile([128, HB * 256], BF16, tag="xtc")
                            nc.sync.dma_start(
                                ch[:].rearrange("w (h c) -> w h c", c=256),
                                x5T[wt, hb * HB:(hb + 1) * HB]
                                .rearrange("h w c -> w h c"))
                            for h in range(HB):
                                nc.tensor.matmul(
                                    pxf[:], fb_sl(wt, hb * HB + h),
                                    ch[:, h * 256:(h + 1) * 256],
                                    start=(mm_i == 0), stop=(mm_i == n_mm - 1))
                                mm_i += 1
                else:
                    # cast owned rows to bf16, bounce through DRAM, read back
                    # transposed as (w, (h c)) slabs
                    HBC = 4
                    sv = act_view(src_t, Ci, src_e)
                    for hb in range(OWN // HBC):
                        cc2 = stm3.tile([128, HBC * 256], BF16, tag="xtc", name="castch")
                        nc.vector.tensor_copy(
                            cc2[:Ci].rearrange("c (h w) -> c h w", w=256),
                            sv[:, src_e + hb * HBC:src_e + (hb + 1) * HBC,
                               1:WP - 1])
                        nc.sync.dma_start(
                            xb_d[tag][hb * HBC:(hb + 1) * HBC]
                            .rearrange("h c w -> c h w"),
                            cc2[:Ci].rearrange("c (h w) -> c h w", w=256))
                    xb_flat = xb_d[tag][:].rearrange("h c w -> (h c) w")
                    HB = 8
                    for wt in range(2):
                        for hb in range(OWN // HB):
                            xt = stm3.tile([128, HB * Ci], BF16, tag="xtc")
                            nc.sync.dma_start_transpose(
                                out=xt[:],
                                in_=xb_flat[hb * HB * Ci:(hb + 1) * HB * Ci,
                                            wt * 128:(wt + 1) * 128])
                            for h in range(HB):
                                nc.tensor.matmul(
                                    pxf[:], fb_sl(wt, hb * HB + h),
                                    xt[:, h * Ci:(h + 1) * Ci],
                                    start=(mm_i == 0), stop=(mm_i == n_mm - 1))
                                mm_i += 1
                xf = tmp1.tile([32, Ci], BF16, tag="xf")
                nc.vector.tensor_copy(xf[:], pxf[:])

                # ---- transpose xf -> xfT (Ci, 32); swapped/negated copy ----
                xfT = tmp1.tile([128, kt * 32], BF16, tag="xfT")
                for k in range(kt):
                    pt = pspec.tile([128, 32], BF16, tag="ptr")
                    w = min(128, Ci - k * 128)
                    nc.tensor.transpose(pt[:w, :], xf[:, k * 128:k * 128 + w],
                                        ident[:32, :32])
                    nc.vector.tensor_copy(xfT[:w, k * 32:(k + 1) * 32],
                                          pt[:w, :])
                xfN = tmp1.tile([128, kt * 32], BF16, tag="xfN")
                xfT3 = xfT[:].rearrange("p (a b) -> p a b", b=2)
                xfN3 = xfN[:].rearrange("p (a b) -> p a b", b=2)
                for k in range(kt):
                    mlo = k * 16
                    nc.scalar.mul(xfN3[:, mlo:mlo + 16, 0:1],
                                  xfT3[:, mlo:mlo + 16, 1:2], -1.0)
                    nc.vector.tensor_copy(xfN3[:, mlo:mlo + 16, 1:2],
                                          xfT3[:, mlo:mlo + 16, 0:1])

                # ---- mix (per mode) -> of (Co, 32) ----
                pof = pspec.tile([Co, 32], F32, tag="pof")
                for mu in range(16):
                    n_grp = 2 * kt
                    gi = 0
                    for k in range(kt):
                        w = min(128, Ci - k * 128)
                        mws = stm3.tile([128, 2 * Co], BF16, tag="mixw")
                        nc.sync.dma_start(
                            mws[:w].rearrange("i (j o) -> i j o", j=2),
                            mixw[tag][mu, :, k * 128:k * 128 + w]
                            .rearrange("j i o -> i j o"))
                        nc.tensor.matmul(
                            pof[:, 2 * mu:2 * mu + 2], mws[:w, :Co],
                            xfT[:w, k * 32 + 2 * mu:k * 32 + 2 * mu + 2],
                            start=(gi == 0), stop=(gi == n_grp - 1))
                        gi += 1
                        nc.tensor.matmul(
                            pof[:, 2 * mu:2 * mu + 2], mws[:w, Co:2 * Co],
                            xfN[:w, k * 32 + 2 * mu:k * 32 + 2 * mu + 2],
                            start=(gi == 0), stop=(gi == n_grp - 1))
                        gi += 1
                of_sb = tmp1.tile([Co, 32], BF16, tag="of_sb")
                nc.vector.tensor_copy(of_sb[:], pof[:])

                # ---- transpose of -> (32, Co); batch-select; AllReduce ----
                pofT = pspec.tile([32, 128], BF16, tag="ptr")
                nc.tensor.transpose(pofT[:, :Co], of_sb[:], ident[:Co, :Co])
                s0 = tmp1.tile([32, Co], F32, tag="sa", name="s0")
                s1 = tmp1.tile([32, Co], F32, tag="sb", name="s1")
                nc.scalar.activation(s0[:], pofT[:, :Co], AF.Copy,
                                     scale=bsel_t[:, 0:1])
                nc.scalar.activation(s1[:], pofT[:, :Co], AF.Copy,
                                     scale=bsel_t[:, 1:2])
                nc.sync.dma_start(cc_in[tag][0], s0[:])
                nc.sync.dma_start(cc_in[tag][1], s1[:])
                nc.gpsimd.collective_compute(
                    "AllReduce", mybir.AluOpType.add, replica_groups=ALL8,
                    ins=[cc_in[tag][:]], outs=[cc_out[tag][:]])
                t0 = tmp1.tile([32, Co], F32, tag="sa", name="t0")
                t1 = tmp1.tile([32, Co], F32, tag="sb", name="t1")
                nc.sync.dma_start(t0[:], cc_out[tag][0])
                nc.sync.dma_start(t1[:], cc_out[tag][1])
                u0 = tmp1.tile([32, Co], F32, tag="u0")
                nc.scalar.activation(u0[:], t0[:], AF.Copy,
                                     scale=bsel_t[:, 0:1])
                u1 = tmp1.tile([32, Co], F32, tag="u1")
                nc.scalar.activation(u1[:], t1[:], AF.Copy,
                                     scale=bsel_t[:, 1:2])
                nc.vector.tensor_add(u0[:], u0[:], u1[:])
                ofb = tmp1.tile([32, Co], BF16, tag="ofb")
                nc.vector.tensor_copy(ofb[:], u0[:])

                # ---- inverse expansion + 1x1 conv + gelu + skip ----
                moff = EMAX - e
                dv = act_view(dst_t, Co, e)
                nr = 1 if tag == "f5" else 2       # rows per chunk
                npix = nr * WW
                for c_i in range(_rows(e) // nr):
                    rd = nr * c_i
                    ps = ppool.tile([Co, npix], F32, tag="cpsum")
                    if tag == "f5":
                        xch = stm2.tile([128, kt * npix], F32R, tag="cwx", name="xch")
                        for k in range(kt):
                            nc.sync.dma_start(
                                xch[:, k * npix:(k + 1) * npix],
                                x5_sl[k * 128:(k + 1) * 128,
                                      rd * WW:(rd + nr) * WW])
                        for k in range(kt):
                            nc.tensor.matmul(
                                ps[:], bwT_t[tag][:, k * Co:(k + 1) * Co],
                                xch[:, k * npix:(k + 1) * npix],
                                start=(k == 0), stop=False)
                    else:
                        sv = act_view(src_t, Ci, src_e)
                        nc.tensor.matmul(
                            ps[:], bwT_t[tag][:Ci, :Co],
                            sv[:, rd:rd + nr, 1:WP - 1],
                            start=True, stop=False)
                    gch = stm2.tile([32, npix], BF16, tag="gbch")
                    nc.sync.dma_start(
                        gch[:], gb[:, (rd + moff) * WW:(rd + moff + nr) * WW])
                    nc.tensor.matmul(ps[:], ofb[:, :Co], gch[:],
                                     start=False, stop=True)
                    dslice = dv[:, rd:rd + nr, 1:WP - 1]
                    nc.scalar.activation(
                        dslice, ps[:].rearrange("c (a w) -> c a w", w=WW),
                        AF.Gelu, bias=bb_t[tag][:Co, 0:1])
                    sk = stm2.tile([Co, npix], F32, tag="skipch")
                    nc.sync.dma_start(sk[:],
                                      skips[tag][:Co, rd * WW:(rd + nr) * WW])
                    nc.vector.tensor_add(
                        dslice, dslice,
                        sk[:].rearrange("c (a w) -> c a w", w=WW))
                band_mask(dst_t, Co, e)

            # ---------------- conv stage ----------------
            def conv_stage(tag, C, st, e, src_t, src_e, dst_t):
                moff = EMAX - e
                wsl = stm2.tile([128, 9 * C], F32R, tag="cwx", name="wsl")
                nc.sync.dma_start(wsl[:C].rearrange("i (t o) -> i t o", t=9),
                                  cw[tag][st].rearrange("t i o -> i t o"))
                sv = act_view(src_t, C, src_e)
                dvv = act_view(dst_t, C, e)
                for c_i in range(_rows(e) // 2):
                    rd = 2 * c_i
                    ps = ppool.tile([C, 512], F32, tag="cpsum")
                    for t9 in range(9):
                        dy, dx = t9 // 3 - 1, t9 % 3 - 1
                        nc.tensor.matmul(
                            ps[:], wsl[:C, t9 * C:(t9 + 1) * C],
                            sv[:, rd + 1 + dy:rd + 3 + dy,
                               1 + dx:WP - 1 + dx],
                            start=(t9 == 0), stop=(t9 == 8))
                    nc.scalar.activation(
                        dvv[:, rd:rd + 2, 1:WP - 1],
                        ps[:].rearrange("c (a w) -> c a w", w=WW), AF.Relu,
                        bias=cb_t[tag][:C, st:st + 1])
                band_mask(dst_t, C, e)

            # ---------------- the network ----------------
            x5u = new_act("A", 128, 9)
            fno_block("f5", 256, 128, 9, None, None, x5u)
            x6a = new_act("B", 128, 8)
            conv_stage("c6", 128, 0, 8, x5u, 9, x6a)
            x6b = new_act("A", 128, 7)
            conv_stage("c6", 128, 1, 7, x6a, 8, x6b)
            x6 = new_act("B", 128, 6)
            conv_stage("c6", 128, 2, 6, x6b, 7, x6)
            x6u = new_act("A", 64, 6)
            fno_block("f6", 128, 64, 6, x6, 6, x6u)
            x7a = new_act("B", 64, 5)
            conv_stage("c7", 64, 0, 5, x6u, 6, x7a)
            x7b = new_act("A", 64, 4)
            conv_stage("c7", 64, 1, 4, x7a, 5, x7b)
            x7 = new_act("B", 64, 3)
            conv_stage("c7", 64, 2, 3, x7b, 4, x7)
            x7u = new_act("A", 32, 3)
            fno_block("f7", 64, 32, 3, x7, 3, x7u)
            x8a = new_act("B", 32, 2)
            conv_stage("c8", 32, 0, 2, x7u, 3, x8a)
            x8 = new_act("A", 32, 1)
            conv_stage("c8", 32, 1, 1, x8a, 2, x8)
            x8u = new_act("B", 16, 1)
            fno_block("f8", 32, 16, 1, x8, 1, x8u)
            x9 = new_act("A", 16, 0)
            conv_stage("c9", 16, 0, 0, x8u, 1, x9)

            # final 1x1 conv (16 -> 2), owned rows only
            x9v = act_view(x9, 16, 0)
            for c_i in range(OWN // 2):
                rd = 2 * c_i
                ps = ppool.tile([2, 512], F32, tag="cpsum")
                nc.tensor.matmul(ps[:], ow_t[:], x9v[:, rd:rd + 2, 1:WP - 1],
                                 start=True, stop=True)
                oc = stm2.tile([2, 512], F32, tag="skipch", name="outch")
                nc.scalar.activation(oc[:], ps[:], AF.Identity, bias=ob_t[:])
                nc.sync.dma_start(out_sl[:, rd * WW:(rd + 2) * WW], oc[:])

    nc.compile()
    return nc


# ---------------------------------------------------------------------------
# host side
# ---------------------------------------------------------------------------

def _slice_rows(x, lo, hi):
    """x: (C, 256, 256) -> (C, hi-lo, 256) zero-padded out of range."""
    C = x.shape[0]
    out = np.zeros((C, hi - lo, WW), np.float32)
    a, b2 = max(lo, 0), min(hi, HH)
    if b2 > a:
        out[:, a - lo:b2 - lo] = x[:, a:b2]
    return out


def _host_inputs(inputs):
    i = inputs
    maps = []
    kk, ll = np.meshgrid(np.arange(M), np.arange(M), indexing="ij")
    kf = kk.reshape(-1).astype(np.float64)   # mu = 4k + l
    lf = ll.reshape(-1).astype(np.float64)
    alpha32 = np.where(lf == 0, 1.0, 2.0).repeat(2)  # per 32-comp row

    def basis(rows_abs, wvals):  # -> (32, len(rows), len(w))
        th = 2 * np.pi * (kf[:, None, None] * rows_abs[None, :, None] / HH
                          + lf[:, None, None] * wvals[None, None, :] / WW)
        out = np.empty((32, len(rows_abs), len(wvals)), np.float32)
        out[0::2] = np.cos(th) / 256.0
        out[1::2] = -np.sin(th) / 256.0
        return out

    # weights (identical on all cores)
    cw_np = {}
    cb_np = {}
    for tag, C, nst, _ in CONVS:
        w = np.asarray(i[tag + "_w"], np.float32)     # (n, co, ci, 3, 3)
        cw_np[tag] = np.ascontiguousarray(
            w.transpose(0, 3, 4, 2, 1).reshape(nst, 9, C, C))
        cbv = np.zeros((128, nst), np.float32)
        cbv[:C] = np.asarray(i[tag + "_b"], np.float32).T
        cb_np[tag] = cbv
    fno_np = {}
    for tag, Ci, Co, e in FNOS:
        kt = (Ci + 127) // 128
        bw = np.asarray(i[tag + "_bw"], np.float32)[:, :, 0, 0]  # (oc, ic)
        full = np.ascontiguousarray(bw.T)                        # (ic, oc)
        bwT_ = np.zeros((128, kt * Co), np.float32)
        for k in range(kt):
            w = min(128, Ci - k * 128)
            bwT_[:w, k * Co:(k + 1) * Co] = full[k * 128:k * 128 + w]
        wr = np.asarray(i[tag + "_wr"], np.float32)   # (ic, oc, 4, 4)
        wi = np.asarray(i[tag + "_wi"], np.float32)
        mw = np.zeros((16, 2, Ci, Co), np.float32)
        for mu in range(16):
            k, l = mu // 4, mu % 4
            mw[mu, 0] = wr[:, :, k, l]
            mw[mu, 1] = wi[:, :, k, l]
        bbv = np.zeros((128, 1), np.float32)
        bbv[:Co, 0] = np.asarray(i[tag + "_bb"], np.float32)
        fno_np[tag] = (bwT_, bbv, mw.astype(ml_dtypes.bfloat16))
    owT_np = np.ascontiguousarray(
        np.asarray(i["ow"], np.float32)[:, :, 0, 0].T)           # (16, 2)
    ob_np = np.asarray(i["ob"], np.float32)[:, None]

    skips_full = {"f5": np.asarray(i["x4"], np.float32),
                  "f6": np.asarray(i["x3"], np.float32),
                  "f7": np.asarray(i["x2"], np.float32),
                  "f8": np.asarray(i["x1"], np.float32)}
    x5 = np.asarray(i["x5"], np.float32)
    wvals = np.arange(WW, dtype=np.float64)
    K_rows9 = _rows(9)

    for core in range(NCORES):
        b, q = divmod(core, 4)
        r0 = OWN * q
        m = {}
        m["x5_sl"] = _slice_rows(x5[b], r0 - 9, r0 + OWN + 9).reshape(256, -1)
        m["x4_sl"] = _slice_rows(skips_full["f5"][b], r0 - 9, r0 + OWN + 9).reshape(128, -1)
        m["x3_sl"] = _slice_rows(skips_full["f6"][b], r0 - 6, r0 + OWN + 6).reshape(64, -1)
        m["x2_sl"] = _slice_rows(skips_full["f7"][b], r0 - 3, r0 + OWN + 3).reshape(32, -1)
        m["x1_sl"] = _slice_rows(skips_full["f8"][b], r0 - 1, r0 + OWN + 1).reshape(16, -1)
        xo = x5[b][:, r0:r0 + OWN, :]                       # (256c, 64h, 256w)
        x5T_ = xo.transpose(1, 2, 0).reshape(OWN, 2, 128, 256).transpose(1, 0, 2, 3)
        m["x5T"] = np.ascontiguousarray(x5T_).astype(ml_dtypes.bfloat16)
        # fwd basis (w, (wt h m)) at abs rows r0+h, col wt*128+w
        fbb = basis(np.arange(r0, r0 + OWN, dtype=np.float64), wvals)  # (32,64,256)
        fbb = (fbb.transpose(2, 1, 0)                       # (w256, h, m)
               .reshape(2, 128, OWN, 32)                    # (wt, w, h, m)
               .transpose(1, 0, 2, 3)                       # (w, wt, h, m)
               .reshape(128, 2 * OWN * 32))
        m["fb"] = np.ascontiguousarray(fbb).astype(ml_dtypes.bfloat16)
        rows = np.arange(r0 - 9, r0 + OWN + 9, dtype=np.float64)
        gbb = basis(rows, wvals) * alpha32[:, None, None]
        gbb[:, (rows < 0) | (rows >= HH), :] = 0.0
        m["gb"] = gbb.reshape(32, -1).astype(ml_dtypes.bfloat16)
        mrow = ((rows >= 0) & (rows < HH)).astype(np.float32)
        m["mask_pp"] = np.tile(mrow[None, :], (128, 1))
        bs = np.zeros((32, 2), np.float32)
        bs[:, b] = 1.0
        m["bsel"] = bs
        m["zpad"] = np.zeros((128, 2 * K_rows9), np.float32)
        for tag, C, nst, _ in CONVS:
            m[tag + "w"] = cw_np[tag]
            m[tag + "b"] = cb_np[tag]
        for tag, Ci, Co, e in FNOS:
            bwT_, bb_, mw_ = fno_np[tag]
            m[tag + "_bwT"] = bwT_
            m[tag + "_bb"] = bb_
            m[tag + "_mixw"] = mw_
        m["owT"] = owT_np
        m["ob"] = ob_np
        maps.append(m)
    return maps


_NC_CACHE = {}


def kernel(**inputs):
    if "nc" not in _NC_CACHE:
        _NC_CACHE["nc"] = _build_nc()
    nc = _NC_CACHE["nc"]
    maps = _host_inputs(inputs)
    res = run_bass_kernel_spmd(nc, maps, list(range(NCORES)), trace=False)
    out = np.zeros((B, 2, HH, WW), np.float32)
    for core in range(NCORES):
        b, q = divmod(core, 4)
        r0 = OWN * q
        out[b, :, r0:r0 + OWN, :] = res.results[core]["out_sl"].reshape(2, OWN, WW)
    return out


# revision 21
# speedup vs baseline: 3.1496x; 1.0489x over previous
"""Trainium2 Bass kernel for nn_Decoder_34325378630277 (FNO-UNet decoder).

Sharding: 8 cores = 2 batches x 4 row-quarters (64 owned rows each).
Conv halos handled by extended recompute (host supplies zero-padded row
slices). FNO spectral path: only 4x4 low modes survive, so the forward
projection is per-channel matmuls against 32 cos/sin basis components
(pixel-major slabs obtained via bf16 DMA transpose through DRAM), the
channel mix is applied to per-core partials (mix commutes with the row
sum), the tiny mixed coefficients are AllReduced, and the inverse
expansion is accumulated directly into the 1x1-conv PSUM. Convs run fp32
(PE rate is dtype-independent); the spectral path runs bf16 (validated
~1e-6 output impact).
"""
import sys, types

sys.path.insert(0, "/opt/trn_rl_repo")
import numpy as np
import ml_dtypes

# NTFF profile hook shim (lets trace=True work under axon; harmless otherwise)
try:
    import antenv  # noqa: F401
    if "antenv.axon_hooks" not in sys.modules:
        _h = {"hook": None}
        _m = types.ModuleType("antenv.axon_hooks")
        _m.set_axon_ntff_profile_hook = lambda h: _h.__setitem__("hook", h)
        _m.get_axon_ntff_profile_hook = lambda: _h["hook"]
        sys.modules["antenv.axon_hooks"] = _m
        from trn_agent_boot.trn_boot import _ntff_profile_via_ctypes
        _m.set_axon_ntff_profile_hook(
            _ntff_profile_via_ctypes("/opt/axon/libaxon_pjrt.so"))
except Exception:
    pass

import concourse.bass as bass
import concourse.bacc as bacc
import concourse.tile as tile
from concourse import mybir, masks
from concourse.bass_utils import run_bass_kernel_spmd

F32 = mybir.dt.float32
F32R = mybir.dt.float32r
BF16 = mybir.dt.bfloat16
AF = mybir.ActivationFunctionType

B, HH, WW, NF = 2, 256, 256, 16
OWN = 64
NCORES = 8
WP = WW + 2          # padded width (zero cols at 0 and WP-1)
EMAX = 9             # x5u halo
M = 4                # modes kept per axis

# fno stages: (tag, Ci, Co, e_dst)
FNOS = [("f5", 256, 128, 9), ("f6", 128, 64, 6), ("f7", 64, 32, 3), ("f8", 32, 16, 1)]
# conv blocks: (tag, C, n_stages, e_dst per stage)
CONVS = [("c6", 128, 3, [8, 7, 6]), ("c7", 64, 3, [5, 4, 3]),
         ("c8", 32, 2, [2, 1]), ("c9", 16, 1, [0])]


def _rows(e):
    return OWN + 2 * e


# ---------------------------------------------------------------------------
# device program
# ---------------------------------------------------------------------------

def _build_nc():
    nc = bacc.Bacc("TRN2", target_bir_lowering=False, debug=False,
                   num_devices=NCORES)

    def din(name, shape, dt):
        return nc.dram_tensor(name, list(shape), dt, kind="ExternalInput").ap()

    # --- external inputs (per-core data) ---
    x5_sl = din("x5_sl", (256, _rows(9) * WW), F32R)
    x4_sl = din("x4_sl", (128, _rows(9) * WW), F32)
    x3_sl = din("x3_sl", (64, _rows(6) * WW), F32)
    x2_sl = din("x2_sl", (32, _rows(3) * WW), F32)
    x1_sl = din("x1_sl", (16, _rows(1) * WW), F32)
    skips = {"f5": x4_sl, "f6": x3_sl, "f7": x2_sl, "f8": x1_sl}
    x5T = din("x5T", (2, OWN, 128, 256), BF16)          # (wt, h, w, c)
    zpad = din("zpad", (128, 2 * _rows(9)), F32R)       # zeros for pad cols
    zpadb = din("zpadb", (128, 2 * _rows(9)), BF16)     # zeros (bf16 buffers)
    fb_in = din("fb", (128, 2 * OWN * 32), BF16)        # (w, (wt h m)) fwd basis
    gb = din("gb", (32, _rows(9) * WW), BF16)           # inv basis rows r0-9..r0+73
    mask_pp = din("mask_pp", (128, _rows(9)), F32)      # in-image mask, replicated
    bsel = din("bsel", (32, 2), F32)                    # one-hot batch select
    CWDT = {"c6": F32R, "c7": F32R, "c8": F32R, "c9": F32R}
    BWDT = {"f5": F32R, "f6": F32R, "f7": F32R, "f8": F32R}
    ACTDT = {"f5": F32R, "f6": BF16, "f7": BF16, "f8": BF16,
             "c6": F32R, "c7": BF16, "c8": BF16, "c9": BF16}
    cw = {}
    cb = {}
    for tag, C, nst, _ in CONVS:
        cw[tag] = din(tag + "w", (nst, 9, C, C), CWDT[tag])
        cb[tag] = din(tag + "b", (128, nst), F32)
    bwT = {}
    bb = {}
    mixw = {}
    for tag, Ci, Co, _ in FNOS:
        kt = (Ci + 127) // 128
        bwT[tag] = din(tag + "_bwT", (128, kt * Co), BWDT[tag])
        bb[tag] = din(tag + "_bb", (128, 1), F32)
        mixw[tag] = din(tag + "_mixw", (16, 2, Ci, Co), BF16)
    owT = din("owT", (16, 2), F32R)
    ob = din("ob", (2, 1), F32)

    out_sl = nc.dram_tensor("out_sl", [2, OWN * WW], F32,
                            kind="ExternalOutput").ap()

    # --- internal dram ---
    xb_d = {}
    for tag, Ci, Co, e in FNOS[1:]:
        xb_d[tag] = nc.dram_tensor("xb_" + tag, [OWN, Ci, 256], BF16).ap()
    cc_in = {}
    cc_out = {}
    for tag, Ci, Co, e in FNOS:
        cc_in[tag] = nc.dram_tensor("ccin_" + tag, [2, 32, Co], F32).ap()
        cc_out[tag] = nc.dram_tensor("ccout_" + tag, [2, 32, Co], F32,
                                     addr_space="Shared").ap()
    ccw_in = nc.dram_tensor("ccw_in", [1, 8], F32).ap()
    ccw_out = nc.dram_tensor("ccw_out", [1, 8], F32, addr_space="Shared").ap()
    ALL8 = [list(range(NCORES))]

    with tile.TileContext(nc) as tc:
        import contextlib
        est = contextlib.ExitStack()
        with est:
            big = est.enter_context(tc.tile_pool(name="big", bufs=1))
            cst = est.enter_context(tc.tile_pool(name="cst", bufs=1))
            stm2 = est.enter_context(tc.tile_pool(name="stm2", bufs=2))
            stm3 = est.enter_context(tc.tile_pool(name="stm3", bufs=3))
            tmp1 = est.enter_context(tc.tile_pool(name="tmp1", bufs=1))
            ppool = est.enter_context(
                tc.tile_pool(name="ppool", bufs=4, space="PSUM"))
            pspec = est.enter_context(
                tc.tile_pool(name="pspec", bufs=1, space="PSUM"))

            # warmup collective (absorbs communicator init early)
            nc.gpsimd.collective_compute(
                "AllReduce", mybir.AluOpType.add, replica_groups=ALL8,
                ins=[ccw_in[:]], outs=[ccw_out[:]])

            # --- constants resident in SBUF ---
            ident = cst.tile([128, 128], BF16, tag="ident")
            masks.make_identity(nc, ident[:])
            mask_t = cst.tile([128, _rows(9)], F32, tag="mask")
            nc.sync.dma_start(mask_t[:], mask_pp[:])
            bsel_t = cst.tile([32, 2], F32, tag="bsel")
            nc.sync.dma_start(bsel_t[:], bsel[:])
            fb_t = cst.tile([128, 2 * OWN * 32], BF16, tag="fb")
            nc.sync.dma_start(fb_t[:], fb_in[:])

            def fb_sl(wt, h):  # (128, 32) lhsT slab
                o = (wt * OWN + h) * 32
                return fb_t[:, o:o + 32]

            bwT_t = {}
            bb_t = {}
            for tag, Ci, Co, e in FNOS:
                kt = (Ci + 127) // 128
                bwT_t[tag] = cst.tile([128, kt * Co], BWDT[tag], tag="bw" + tag, name="bw_" + tag)
                nc.sync.dma_start(bwT_t[tag][:], bwT[tag][:])
                bb_t[tag] = cst.tile([128, 1], F32, tag="bb" + tag, name="bbt_" + tag)
                nc.sync.dma_start(bb_t[tag][:], bb[tag][:])
            cb_t = {}
            for tag, C, nst, _ in CONVS:
                cb_t[tag] = cst.tile([128, nst], F32, tag="cb" + tag, name="cbt_" + tag)
                nc.sync.dma_start(cb_t[tag][:], cb[tag][:])
            ow_t = cst.tile([16, 2], F32R, tag="ow")
            nc.sync.dma_start(ow_t[:], owT[:])
            ob_t = cst.tile([2, 1], F32, tag="ob")
            nc.sync.dma_start(ob_t[:], ob[:])

            # --- big activation slots ---
            def new_act(slot, C, e, dt=F32R):
                t = big.tile([128, _rows(e) * WP], dt, tag=slot, name="act_" + slot)
                R = _rows(e)
                zsrc = zpad if dt == F32R else zpadb
                # zero the pad columns (cols 0 and WP-1 of every row)
                z = t[:].rearrange("c (r w) -> c r w", w=WP)
                nc.sync.dma_start(z[:, :, 0:1], zsrc[:, :R])
                nc.sync.dma_start(z[:, :, WP - 1:WP], zsrc[:, R:2 * R])
                return t

            def act_view(t, C, e):
                return t[:C].rearrange("c (r w) -> c r w", w=WP)

            def band_mask(dst_t, C, e):
                moff = EMAX - e
                R = _rows(e)
                dv = act_view(dst_t, C, e)
                for lo in (0, R - 9):
                    nc.vector.tensor_mul(
                        dv[:, lo:lo + 9, :], dv[:, lo:lo + 9, :],
                        mask_t[:C, moff + lo:moff + lo + 9]
                        .broadcast_to((C, 9, WP)))

            # ---------------- fno block ----------------
            def fno_block(tag, Ci, Co, e, src_t, src_e, dst_t):
                kt = (Ci + 127) // 128
                # ---- forward projection -> partial xf (32, Ci) ----
                pxf = pspec.tile([32, Ci], F32, tag="pxf")
                n_mm = 2 * OWN
                mm_i = 0
                if tag == "f5":
                    HB = 4
                    for wt in range(2):
                        for hb in range(OWN // HB):
                            ch = stm3.tile([128, HB * 256], BF16, tag="xtc")
                            nc.sync.dma_start(
                                ch[:].rearrange("w (h c) -> w h c", c=256),
                                x5T[wt, hb * HB:(hb + 1) * HB]
                                .rearrange("h w c -> w h c"))
                            for h in range(HB):
                                nc.tensor.matmul(
                                    pxf[:], fb_sl(wt, hb * HB + h),
                                    ch[:, h * 256:(h + 1) * 256],
                                    start=(mm_i == 0), stop=(mm_i == n_mm - 1))
                                mm_i += 1
                else:
                    # cast owned rows to bf16, bounce through DRAM, read back
                    # transposed as (w, (h c)) slabs
                    HBC = 4
                    sv = act_view(src_t, Ci, src_e)
                    for hb in range(OWN // HBC):
                        cc2 = stm3.tile([128, HBC * 256], BF16, tag="xtc", name="castch")
                        nc.vector.tensor_copy(
                            cc2[:Ci].rearrange("c (h w) -> c h w", w=256),
                            sv[:, src_e + hb * HBC:src_e + (hb + 1) * HBC,
                               1:WP - 1])
                        nc.sync.dma_start(
                            xb_d[tag][hb * HBC:(hb + 1) * HBC]
                            .rearrange("h c w -> c h w"),
                            cc2[:Ci].rearrange("c (h w) -> c h w", w=256))
                    xb_flat = xb_d[tag][:].rearrange("h c w -> (h c) w")
                    HB = 8
                    for wt in range(2):
                        for hb in range(OWN // HB):
                            xt = stm3.tile([128, HB * Ci], BF16, tag="xtc")
                            nc.sync.dma_start_transpose(
                                out=xt[:],
                                in_=xb_flat[hb * HB * Ci:(hb + 1) * HB * Ci,
                                            wt * 128:(wt + 1) * 128])
                            for h in range(HB):
                                nc.tensor.matmul(
                                    pxf[:], fb_sl(wt, hb * HB + h),
                                    xt[:, h * Ci:(h + 1) * Ci],
                                    start=(mm_i == 0), stop=(mm_i == n_mm - 1))
                                mm_i += 1
                xf = tmp1.tile([32, Ci], BF16, tag="xf")
                nc.vector.tensor_copy(xf[:], pxf[:])

                # ---- transpose xf -> xfT (Ci, 32); swapped/negated copy ----
                xfT = tmp1.tile([128, kt * 32], BF16, tag="xfT")
                for k in range(kt):
                    pt = pspec.tile([128, 32], BF16, tag="ptr")
                    w = min(128, Ci - k * 128)
                    nc.tensor.transpose(pt[:w, :], xf[:, k * 128:k * 128 + w],
                                        ident[:32, :32])
                    nc.vector.tensor_copy(xfT[:w, k * 32:(k + 1) * 32],
                                          pt[:w, :])
                xfN = tmp1.tile([128, kt * 32], BF16, tag="xfN")
                xfT3 = xfT[:].rearrange("p (a b) -> p a b", b=2)
                xfN3 = xfN[:].rearrange("p (a b) -> p a b", b=2)
                for k in range(kt):
                    mlo = k * 16
                    nc.scalar.mul(xfN3[:, mlo:mlo + 16, 0:1],
                                  xfT3[:, mlo:mlo + 16, 1:2], -1.0)
                    nc.vector.tensor_copy(xfN3[:, mlo:mlo + 16, 1:2],
                                          xfT3[:, mlo:mlo + 16, 0:1])

                # ---- mix (per mode) -> of (Co, 32) ----
                pof = pspec.tile([Co, 32], F32, tag="pof")
                for mu in range(16):
                    n_grp = 2 * kt
                    gi = 0
                    for k in range(kt):
                        w = min(128, Ci - k * 128)
                        mws = stm3.tile([128, 2 * Co], BF16, tag="mixw")
                        nc.sync.dma_start(
                            mws[:w].rearrange("i (j o) -> i j o", j=2),
                            mixw[tag][mu, :, k * 128:k * 128 + w]
                            .rearrange("j i o -> i j o"))
                        nc.tensor.matmul(
                            pof[:, 2 * mu:2 * mu + 2], mws[:w, :Co],
                            xfT[:w, k * 32 + 2 * mu:k * 32 + 2 * mu + 2],
                            start=(gi == 0), stop=(gi == n_grp - 1))
                        gi += 1
                        nc.tensor.matmul(
                            pof[:, 2 * mu:2 * mu + 2], mws[:w, Co:2 * Co],
                            xfN[:w, k * 32 + 2 * mu:k * 32 + 2 * mu + 2],
                            start=(gi == 0), stop=(gi == n_grp - 1))
                        gi += 1
                of_sb = tmp1.tile([Co, 32], BF16, tag="of_sb")
                nc.vector.tensor_copy(of_sb[:], pof[:])

                # ---- transpose of -> (32, Co); batch-select; AllReduce ----
                pofT = pspec.tile([32, 128], BF16, tag="ptr")
                nc.tensor.transpose(pofT[:, :Co], of_sb[:], ident[:Co, :Co])
                s0 = tmp1.tile([32, Co], F32, tag="sa", name="s0")
                s1 = tmp1.tile([32, Co], F32, tag="sb", name="s1")
                nc.scalar.activation(s0[:], pofT[:, :Co], AF.Copy,
                                     scale=bsel_t[:, 0:1])
                nc.scalar.activation(s1[:], pofT[:, :Co], AF.Copy,
                                     scale=bsel_t[:, 1:2])
                nc.sync.dma_start(cc_in[tag][0], s0[:])
                nc.sync.dma_start(cc_in[tag][1], s1[:])
                nc.gpsimd.collective_compute(
                    "AllReduce", mybir.AluOpType.add, replica_groups=ALL8,
                    ins=[cc_in[tag][:]], outs=[cc_out[tag][:]])
                t0 = tmp1.tile([32, Co], F32, tag="sa", name="t0")
                t1 = tmp1.tile([32, Co], F32, tag="sb", name="t1")
                nc.sync.dma_start(t0[:], cc_out[tag][0])
                nc.sync.dma_start(t1[:], cc_out[tag][1])
                u0 = tmp1.tile([32, Co], F32, tag="u0")
                nc.scalar.activation(u0[:], t0[:], AF.Copy,
                                     scale=bsel_t[:, 0:1])
                u1 = tmp1.tile([32, Co], F32, tag="u1")
                nc.scalar.activation(u1[:], t1[:], AF.Copy,
                                     scale=bsel_t[:, 1:2])
                nc.vector.tensor_add(u0[:], u0[:], u1[:])
                ofb = tmp1.tile([32, Co], BF16, tag="ofb")
                nc.vector.tensor_copy(ofb[:], u0[:])

                # ---- inverse expansion + 1x1 conv + gelu + skip ----
                moff = EMAX - e
                dv = act_view(dst_t, Co, e)
                nr = 2                             # rows per chunk
                npix = nr * WW
                for c_i in range(_rows(e) // nr):
                    rd = nr * c_i
                    ps = ppool.tile([Co, npix], F32, tag="cpsum")
                    if tag == "f5":
                        xch = stm2.tile([128, kt * npix], F32R, tag="cwx", name="xch")
                        for k in range(kt):
                            nc.sync.dma_start(
                                xch[:, k * npix:(k + 1) * npix],
                                x5_sl[k * 128:(k + 1) * 128,
                                      rd * WW:(rd + nr) * WW])
                        for k in range(kt):
                            nc.tensor.matmul(
                                ps[:], bwT_t[tag][:, k * Co:(k + 1) * Co],
                                xch[:, k * npix:(k + 1) * npix],
                                start=(k == 0), stop=False)
                    else:
                        sv = act_view(src_t, Ci, src_e)
                        nc.tensor.matmul(
                            ps[:], bwT_t[tag][:Ci, :Co],
                            sv[:, rd:rd + nr, 1:WP - 1],
                            start=True, stop=False)
                    gch = stm2.tile([32, npix], BF16, tag="gbch")
                    nc.sync.dma_start(
                        gch[:], gb[:, (rd + moff) * WW:(rd + moff + nr) * WW])
                    nc.tensor.matmul(ps[:], ofb[:, :Co], gch[:],
                                     start=False, stop=True)
                    dslice = dv[:, rd:rd + nr, 1:WP - 1]
                    nc.scalar.activation(
                        dslice, ps[:].rearrange("c (a w) -> c a w", w=WW),
                        AF.Gelu, bias=bb_t[tag][:Co, 0:1])
                    sk = stm2.tile([Co, npix], F32, tag="skipch")
                    nc.sync.dma_start(sk[:],
                                      skips[tag][:Co, rd * WW:(rd + nr) * WW])
                    nc.vector.tensor_add(
                        dslice, dslice,
                        sk[:].rearrange("c (a w) -> c a w", w=WW))
                band_mask(dst_t, Co, e)

            # ---------------- conv stage ----------------
            def conv_stage(tag, C, st, e, src_t, src_e, dst_t):
                moff = EMAX - e
                wsl = stm2.tile([128, 9 * C], CWDT[tag], tag="cwx", name="wsl")
                nc.sync.dma_start(wsl[:C].rearrange("i (t o) -> i t o", t=9),
                                  cw[tag][st].rearrange("t i o -> i t o"))
                sv = act_view(src_t, C, src_e)
                dvv = act_view(dst_t, C, e)
                for c_i in range(_rows(e) // 2):
                    rd = 2 * c_i
                    ps = ppool.tile([C, 512], F32, tag="cpsum")
                    for t9 in range(9):
                        dy, dx = t9 // 3 - 1, t9 % 3 - 1
                        nc.tensor.matmul(
                            ps[:], wsl[:C, t9 * C:(t9 + 1) * C],
                            sv[:, rd + 1 + dy:rd + 3 + dy,
                               1 + dx:WP - 1 + dx],
                            start=(t9 == 0), stop=(t9 == 8))
                    nc.scalar.activation(
                        dvv[:, rd:rd + 2, 1:WP - 1],
                        ps[:].rearrange("c (a w) -> c a w", w=WW), AF.Relu,
                        bias=cb_t[tag][:C, st:st + 1])
                band_mask(dst_t, C, e)

            # ---------------- the network ----------------
            x5u = new_act("A", 128, 9)
            fno_block("f5", 256, 128, 9, None, None, x5u)
            x6a = new_act("B", 128, 8)
            conv_stage("c6", 128, 0, 8, x5u, 9, x6a)
            x6b = new_act("A", 128, 7)
            conv_stage("c6", 128, 1, 7, x6a, 8, x6b)
            x6 = new_act("B", 128, 6)
            conv_stage("c6", 128, 2, 6, x6b, 7, x6)
            x6u = new_act("A", 64, 6)
            fno_block("f6", 128, 64, 6, x6, 6, x6u)
            x7a = new_act("B", 64, 5)
            conv_stage("c7", 64, 0, 5, x6u, 6, x7a)
            x7b = new_act("A", 64, 4)
            conv_stage("c7", 64, 1, 4, x7a, 5, x7b)
            x7 = new_act("B", 64, 3)
            conv_stage("c7", 64, 2, 3, x7b, 4, x7)
            x7u = new_act("A", 32, 3)
            fno_block("f7", 64, 32, 3, x7, 3, x7u)
            x8a = new_act("B", 32, 2)
            conv_stage("c8", 32, 0, 2, x7u, 3, x8a)
            x8 = new_act("A", 32, 1)
            conv_stage("c8", 32, 1, 1, x8a, 2, x8)
            x8u = new_act("B", 16, 1)
            fno_block("f8", 32, 16, 1, x8, 1, x8u)
            x9 = new_act("A", 16, 0)
            conv_stage("c9", 16, 0, 0, x8u, 1, x9)

            # final 1x1 conv (16 -> 2), owned rows only
            x9v = act_view(x9, 16, 0)
            for c_i in range(OWN // 2):
                rd = 2 * c_i
                ps = ppool.tile([2, 512], F32, tag="cpsum")
                nc.tensor.matmul(ps[:], ow_t[:], x9v[:, rd:rd + 2, 1:WP - 1],
                                 start=True, stop=True)
                oc = stm2.tile([2, 512], F32, tag="skipch", name="outch")
                nc.scalar.activation(oc[:], ps[:], AF.Identity, bias=ob_t[:])
                nc.sync.dma_start(out_sl[:, rd * WW:(rd + 2) * WW], oc[:])

    nc.compile()
    return nc


# ---------------------------------------------------------------------------
# host side
# ---------------------------------------------------------------------------

def _slice_rows(x, lo, hi):
    """x: (C, 256, 256) -> (C, hi-lo, 256) zero-padded out of range."""
    C = x.shape[0]
    out = np.zeros((C, hi - lo, WW), np.float32)
    a, b2 = max(lo, 0), min(hi, HH)
    if b2 > a:
        out[:, a - lo:b2 - lo] = x[:, a:b2]
    return out


def _host_inputs(inputs):
    i = inputs
    maps = []
    kk, ll = np.meshgrid(np.arange(M), np.arange(M), indexing="ij")
    kf = kk.reshape(-1).astype(np.float64)   # mu = 4k + l
    lf = ll.reshape(-1).astype(np.float64)
    alpha32 = np.where(lf == 0, 1.0, 2.0).repeat(2)  # per 32-comp row

    def basis(rows_abs, wvals):  # -> (32, len(rows), len(w))
        th = 2 * np.pi * (kf[:, None, None] * rows_abs[None, :, None] / HH
                          + lf[:, None, None] * wvals[None, None, :] / WW)
        out = np.empty((32, len(rows_abs), len(wvals)), np.float32)
        out[0::2] = np.cos(th) / 256.0
        out[1::2] = -np.sin(th) / 256.0
        return out

    # weights (identical on all cores)
    cw_np = {}
    cb_np = {}
    for tag, C, nst, _ in CONVS:
        w = np.asarray(i[tag + "_w"], np.float32)     # (n, co, ci, 3, 3)
        cwv = np.ascontiguousarray(
            w.transpose(0, 3, 4, 2, 1).reshape(nst, 9, C, C))
        cw_np[tag] = cwv
        cbv = np.zeros((128, nst), np.float32)
        cbv[:C] = np.asarray(i[tag + "_b"], np.float32).T
        cb_np[tag] = cbv
    fno_np = {}
    for tag, Ci, Co, e in FNOS:
        kt = (Ci + 127) // 128
        bw = np.asarray(i[tag + "_bw"], np.float32)[:, :, 0, 0]  # (oc, ic)
        full = np.ascontiguousarray(bw.T)                        # (ic, oc)
        bwT_ = np.zeros((128, kt * Co), np.float32)
        for k in range(kt):
            w = min(128, Ci - k * 128)
            bwT_[:w, k * Co:(k + 1) * Co] = full[k * 128:k * 128 + w]
        wr = np.asarray(i[tag + "_wr"], np.float32)   # (ic, oc, 4, 4)
        wi = np.asarray(i[tag + "_wi"], np.float32)
        mw = np.zeros((16, 2, Ci, Co), np.float32)
        for mu in range(16):
            k, l = mu // 4, mu % 4
            mw[mu, 0] = wr[:, :, k, l]
            mw[mu, 1] = wi[:, :, k, l]
        bbv = np.zeros((128, 1), np.float32)
        bbv[:Co, 0] = np.asarray(i[tag + "_bb"], np.float32)
        fno_np[tag] = (bwT_, bbv, mw.astype(ml_dtypes.bfloat16))
    owT_np = np.ascontiguousarray(
        np.asarray(i["ow"], np.float32)[:, :, 0, 0].T)           # (16, 2)
    ob_np = np.asarray(i["ob"], np.float32)[:, None]

    skips_full = {"f5": np.asarray(i["x4"], np.float32),
                  "f6": np.asarray(i["x3"], np.float32),
                  "f7": np.asarray(i["x2"], np.float32),
                  "f8": np.asarray(i["x1"], np.float32)}
    x5 = np.asarray(i["x5"], np.float32)
    wvals = np.arange(WW, dtype=np.float64)
    K_rows9 = _rows(9)

    for core in range(NCORES):
        b, q = divmod(core, 4)
        r0 = OWN * q
        m = {}
        m["x5_sl"] = _slice_rows(x5[b], r0 - 9, r0 + OWN + 9).reshape(256, -1)
        m["x4_sl"] = _slice_rows(skips_full["f5"][b], r0 - 9, r0 + OWN + 9).reshape(128, -1)
        m["x3_sl"] = _slice_rows(skips_full["f6"][b], r0 - 6, r0 + OWN + 6).reshape(64, -1)
        m["x2_sl"] = _slice_rows(skips_full["f7"][b], r0 - 3, r0 + OWN + 3).reshape(32, -1)
        m["x1_sl"] = _slice_rows(skips_full["f8"][b], r0 - 1, r0 + OWN + 1).reshape(16, -1)
        xo = x5[b][:, r0:r0 + OWN, :]                       # (256c, 64h, 256w)
        x5T_ = xo.transpose(1, 2, 0).reshape(OWN, 2, 128, 256).transpose(1, 0, 2, 3)
        m["x5T"] = np.ascontiguousarray(x5T_).astype(ml_dtypes.bfloat16)
        # fwd basis (w, (wt h m)) at abs rows r0+h, col wt*128+w
        fbb = basis(np.arange(r0, r0 + OWN, dtype=np.float64), wvals)  # (32,64,256)
        fbb = (fbb.transpose(2, 1, 0)                       # (w256, h, m)
               .reshape(2, 128, OWN, 32)                    # (wt, w, h, m)
               .transpose(1, 0, 2, 3)                       # (w, wt, h, m)
               .reshape(128, 2 * OWN * 32))
        m["fb"] = np.ascontiguousarray(fbb).astype(ml_dtypes.bfloat16)
        rows = np.arange(r0 - 9, r0 + OWN + 9, dtype=np.float64)
        gbb = basis(rows, wvals) * alpha32[:, None, None]
        gbb[:, (rows < 0) | (rows >= HH), :] = 0.0
        m["gb"] = gbb.reshape(32, -1).astype(ml_dtypes.bfloat16)
        mrow = ((rows >= 0) & (rows < HH)).astype(np.float32)
        m["mask_pp"] = np.tile(mrow[None, :], (128, 1))
        bs = np.zeros((32, 2), np.float32)
        bs[:, b] = 1.0
        m["bsel"] = bs
        m["zpad"] = np.zeros((128, 2 * K_rows9), np.float32)
        m["zpadb"] = np.zeros((128, 2 * K_rows9), ml_dtypes.bfloat16)
        for tag, C, nst, _ in CONVS:
            m[tag + "w"] = cw_np[tag]
            m[tag + "b"] = cb_np[tag]
        for tag, Ci, Co, e in FNOS:
            bwT_, bb_, mw_ = fno_np[tag]
            m[tag + "_bwT"] = bwT_
            m[tag + "_bb"] = bb_
            m[tag + "_mixw"] = mw_
        m["owT"] = owT_np
        m["ob"] = ob_np
        maps.append(m)
    return maps


_NC_CACHE = {}


def kernel(**inputs):
    if "nc" not in _NC_CACHE:
        _NC_CACHE["nc"] = _build_nc()
    nc = _NC_CACHE["nc"]
    maps = _host_inputs(inputs)
    res = run_bass_kernel_spmd(nc, maps, list(range(NCORES)), trace=False)
    out = np.zeros((B, 2, HH, WW), np.float32)
    for core in range(NCORES):
        b, q = divmod(core, 4)
        r0 = OWN * q
        out[b, :, r0:r0 + OWN, :] = res.results[core]["out_sl"].reshape(2, OWN, WW)
    return out


# revision 22
# speedup vs baseline: 3.6876x; 1.1708x over previous
"""Trainium2 Bass kernel for nn_Decoder_34325378630277 (FNO-UNet decoder).

Sharding: 8 cores = 2 batches x 4 row-quarters (64 owned rows each).
Conv halos handled by extended recompute (host supplies zero-padded row
slices). FNO spectral path: only 4x4 low modes survive, so the forward
projection is per-channel matmuls against 32 cos/sin basis components
(pixel-major slabs obtained via bf16 DMA transpose through DRAM), the
channel mix is applied to per-core partials (mix commutes with the row
sum), the tiny mixed coefficients are AllReduced, and the inverse
expansion is accumulated directly into the 1x1-conv PSUM. Convs run fp32
(PE rate is dtype-independent); the spectral path runs bf16 (validated
~1e-6 output impact).
"""
import sys, types

sys.path.insert(0, "/opt/trn_rl_repo")
import numpy as np
import ml_dtypes

# NTFF profile hook shim (lets trace=True work under axon; harmless otherwise)
try:
    import antenv  # noqa: F401
    if "antenv.axon_hooks" not in sys.modules:
        _h = {"hook": None}
        _m = types.ModuleType("antenv.axon_hooks")
        _m.set_axon_ntff_profile_hook = lambda h: _h.__setitem__("hook", h)
        _m.get_axon_ntff_profile_hook = lambda: _h["hook"]
        sys.modules["antenv.axon_hooks"] = _m
        from trn_agent_boot.trn_boot import _ntff_profile_via_ctypes
        _m.set_axon_ntff_profile_hook(
            _ntff_profile_via_ctypes("/opt/axon/libaxon_pjrt.so"))
except Exception:
    pass

import concourse.bass as bass
import concourse.bacc as bacc
import concourse.tile as tile
from concourse import mybir, masks
from concourse.bass_utils import run_bass_kernel_spmd

F32 = mybir.dt.float32
F32R = mybir.dt.float32r
BF16 = mybir.dt.bfloat16
AF = mybir.ActivationFunctionType

B, HH, WW, NF = 2, 256, 256, 16
OWN = 64
NCORES = 8
WP = WW + 2          # padded width (zero cols at 0 and WP-1)
EMAX = 9             # x5u halo
M = 4                # modes kept per axis

# fno stages: (tag, Ci, Co, e_dst)
FNOS = [("f5", 256, 128, 9), ("f6", 128, 64, 6), ("f7", 64, 32, 3), ("f8", 32, 16, 1)]
# conv blocks: (tag, C, n_stages, e_dst per stage)
CONVS = [("c6", 128, 3, [8, 7, 6]), ("c7", 64, 3, [5, 4, 3]),
         ("c8", 32, 2, [2, 1]), ("c9", 16, 1, [0])]


def _rows(e):
    return OWN + 2 * e


# ---------------------------------------------------------------------------
# device program
# ---------------------------------------------------------------------------

def _build_nc():
    nc = bacc.Bacc("TRN2", target_bir_lowering=False, debug=False,
                   num_devices=NCORES)

    def din(name, shape, dt):
        return nc.dram_tensor(name, list(shape), dt, kind="ExternalInput").ap()

    # --- external inputs (per-core data) ---
    x5_sl = din("x5_sl", (256, _rows(9) * WW), F32R)
    x4_sl = din("x4_sl", (128, _rows(9) * WW), F32)
    x3_sl = din("x3_sl", (64, _rows(6) * WW), F32)
    x2_sl = din("x2_sl", (32, _rows(3) * WW), F32)
    x1_sl = din("x1_sl", (16, _rows(1) * WW), F32)
    skips = {"f5": x4_sl, "f6": x3_sl, "f7": x2_sl, "f8": x1_sl}
    x5T = din("x5T", (2, OWN, 128, 256), BF16)          # (wt, h, w, c)
    zpad = din("zpad", (128, 2 * _rows(9)), F32R)       # zeros for pad cols
    zpadb = din("zpadb", (128, 2 * _rows(9)), BF16)     # zeros (bf16 buffers)
    fb_in = din("fb", (128, 2 * OWN * 32), BF16)        # (w, (wt h m)) fwd basis
    gb = din("gb", (32, _rows(9) * WW), BF16)           # inv basis rows r0-9..r0+73
    mask_pp = din("mask_pp", (128, _rows(9)), F32)      # in-image mask, replicated
    bsel = din("bsel", (32, 2), F32)                    # one-hot batch select
    CWDT = {"c6": F32R, "c7": F32R, "c8": F32R, "c9": F32R}
    BWDT = {"f5": F32R, "f6": F32R, "f7": F32R, "f8": F32R}
    ACTDT = {"f5": F32R, "f6": BF16, "f7": BF16, "f8": BF16,
             "c6": F32R, "c7": BF16, "c8": BF16, "c9": BF16}
    CWSHP = {"c6": (3, 9, 128, 128), "c7": (3, 6, 128, 64),
             "c8": (2, 3, 96, 32), "c9": (1, 3, 48, 16)}
    cw = {}
    cb = {}
    for tag, C, nst, _ in CONVS:
        cw[tag] = din(tag + "w", CWSHP[tag], CWDT[tag])
        cb[tag] = din(tag + "b", (128, nst), F32)
    bwT = {}
    bb = {}
    mixw = {}
    for tag, Ci, Co, _ in FNOS:
        kt = (Ci + 127) // 128
        bwT[tag] = din(tag + "_bwT", (128, kt * Co), BWDT[tag])
        bb[tag] = din(tag + "_bb", (128, 1), F32)
        mixw[tag] = din(tag + "_mixw", (16, 2, Ci, Co), BF16)
    owT = din("owT", (16, 2), F32R)
    ob = din("ob", (2, 1), F32)

    out_sl = nc.dram_tensor("out_sl", [2, OWN * WW], F32,
                            kind="ExternalOutput").ap()

    # --- internal dram ---
    xb_d = {}
    for tag, Ci, Co, e in FNOS[1:]:
        xb_d[tag] = nc.dram_tensor("xb_" + tag, [OWN, Ci, 256], BF16).ap()
    cc_in = {}
    cc_out = {}
    for tag, Ci, Co, e in FNOS:
        cc_in[tag] = nc.dram_tensor("ccin_" + tag, [2, 32, Co], F32).ap()
        cc_out[tag] = nc.dram_tensor("ccout_" + tag, [2, 32, Co], F32,
                                     addr_space="Shared").ap()
    ccw_in = nc.dram_tensor("ccw_in", [1, 8], F32).ap()
    ccw_out = nc.dram_tensor("ccw_out", [1, 8], F32, addr_space="Shared").ap()
    ALL8 = [list(range(NCORES))]

    with tile.TileContext(nc) as tc:
        import contextlib
        est = contextlib.ExitStack()
        with est:
            big = est.enter_context(tc.tile_pool(name="big", bufs=1))
            cst = est.enter_context(tc.tile_pool(name="cst", bufs=1))
            stm2 = est.enter_context(tc.tile_pool(name="stm2", bufs=2))
            stm3 = est.enter_context(tc.tile_pool(name="stm3", bufs=3))
            tmp1 = est.enter_context(tc.tile_pool(name="tmp1", bufs=1))
            ppool = est.enter_context(
                tc.tile_pool(name="ppool", bufs=4, space="PSUM"))
            pspec = est.enter_context(
                tc.tile_pool(name="pspec", bufs=1, space="PSUM"))

            # warmup collective (absorbs communicator init early)
            nc.gpsimd.collective_compute(
                "AllReduce", mybir.AluOpType.add, replica_groups=ALL8,
                ins=[ccw_in[:]], outs=[ccw_out[:]])

            # --- constants resident in SBUF ---
            ident = cst.tile([128, 128], BF16, tag="ident")
            masks.make_identity(nc, ident[:])
            mask_t = cst.tile([128, _rows(9)], F32, tag="mask")
            nc.sync.dma_start(mask_t[:], mask_pp[:])
            bsel_t = cst.tile([32, 2], F32, tag="bsel")
            nc.sync.dma_start(bsel_t[:], bsel[:])
            fb_t = cst.tile([128, 2 * OWN * 32], BF16, tag="fb")
            nc.sync.dma_start(fb_t[:], fb_in[:])

            def fb_sl(wt, h):  # (128, 32) lhsT slab
                o = (wt * OWN + h) * 32
                return fb_t[:, o:o + 32]

            bwT_t = {}
            bb_t = {}
            for tag, Ci, Co, e in FNOS:
                kt = (Ci + 127) // 128
                bwT_t[tag] = cst.tile([128, kt * Co], BWDT[tag], tag="bw" + tag, name="bw_" + tag)
                nc.sync.dma_start(bwT_t[tag][:], bwT[tag][:])
                bb_t[tag] = cst.tile([128, 1], F32, tag="bb" + tag, name="bbt_" + tag)
                nc.sync.dma_start(bb_t[tag][:], bb[tag][:])
            cb_t = {}
            for tag, C, nst, _ in CONVS:
                cb_t[tag] = cst.tile([128, nst], F32, tag="cb" + tag, name="cbt_" + tag)
                nc.sync.dma_start(cb_t[tag][:], cb[tag][:])
            ow_t = cst.tile([16, 2], F32R, tag="ow")
            nc.sync.dma_start(ow_t[:], owT[:])
            ob_t = cst.tile([2, 1], F32, tag="ob")
            nc.sync.dma_start(ob_t[:], ob[:])

            # --- big activation slots ---
            def new_act(slot, C, e, dt=F32R):
                t = big.tile([128, _rows(e) * WP], dt, tag=slot, name="act_" + slot)
                R = _rows(e)
                zsrc = zpad if dt == F32R else zpadb
                # zero the pad columns (cols 0 and WP-1 of every row)
                z = t[:].rearrange("c (r w) -> c r w", w=WP)
                nc.sync.dma_start(z[:, :, 0:1], zsrc[:, :R])
                nc.sync.dma_start(z[:, :, WP - 1:WP], zsrc[:, R:2 * R])
                return t

            def act_view(t, C, e):
                return t[:C].rearrange("c (r w) -> c r w", w=WP)

            def stack_copies(t, C, e, nstack):
                R = _rows(e)
                v = t[:].rearrange("c (r w) -> c r w", w=WP)
                for k in range(1, nstack):
                    nc.sync.dma_start(v[k * C:(k + 1) * C, 0:R - k, :],
                                      v[0:C, k:R, :])

            def band_mask(dst_t, C, e):
                moff = EMAX - e
                R = _rows(e)
                dv = act_view(dst_t, C, e)
                for lo in (0, R - 9):
                    nc.vector.tensor_mul(
                        dv[:, lo:lo + 9, :], dv[:, lo:lo + 9, :],
                        mask_t[:C, moff + lo:moff + lo + 9]
                        .broadcast_to((C, 9, WP)))

            # ---------------- fno block ----------------
            def fno_block(tag, Ci, Co, e, src_t, src_e, dst_t,
                          dst_stack=1):
                kt = (Ci + 127) // 128
                # ---- forward projection -> partial xf (32, Ci) ----
                pxf = pspec.tile([32, Ci], F32, tag="pxf")
                n_mm = 2 * OWN
                mm_i = 0
                if tag == "f5":
                    HB = 4
                    for wt in range(2):
                        for hb in range(OWN // HB):
                            ch = stm3.tile([128, HB * 256], BF16, tag="xtc")
                            nc.sync.dma_start(
                                ch[:].rearrange("w (h c) -> w h c", c=256),
                                x5T[wt, hb * HB:(hb + 1) * HB]
                                .rearrange("h w c -> w h c"))
                            for h in range(HB):
                                nc.tensor.matmul(
                                    pxf[:], fb_sl(wt, hb * HB + h),
                                    ch[:, h * 256:(h + 1) * 256],
                                    start=(mm_i == 0), stop=(mm_i == n_mm - 1))
                                mm_i += 1
                else:
                    # cast owned rows to bf16, bounce through DRAM, read back
                    # transposed as (w, (h c)) slabs
                    HBC = 4
                    sv = act_view(src_t, Ci, src_e)
                    for hb in range(OWN // HBC):
                        cc2 = stm3.tile([128, HBC * 256], BF16, tag="xtc", name="castch")
                        nc.vector.tensor_copy(
                            cc2[:Ci].rearrange("c (h w) -> c h w", w=256),
                            sv[:, src_e + hb * HBC:src_e + (hb + 1) * HBC,
                               1:WP - 1])
                        nc.sync.dma_start(
                            xb_d[tag][hb * HBC:(hb + 1) * HBC]
                            .rearrange("h c w -> c h w"),
                            cc2[:Ci].rearrange("c (h w) -> c h w", w=256))
                    xb_flat = xb_d[tag][:].rearrange("h c w -> (h c) w")
                    HB = 8
                    for wt in range(2):
                        for hb in range(OWN // HB):
                            xt = stm3.tile([128, HB * Ci], BF16, tag="xtc")
                            nc.sync.dma_start_transpose(
                                out=xt[:],
                                in_=xb_flat[hb * HB * Ci:(hb + 1) * HB * Ci,
                                            wt * 128:(wt + 1) * 128])
                            for h in range(HB):
                                nc.tensor.matmul(
                                    pxf[:], fb_sl(wt, hb * HB + h),
                                    xt[:, h * Ci:(h + 1) * Ci],
                                    start=(mm_i == 0), stop=(mm_i == n_mm - 1))
                                mm_i += 1
                xf = tmp1.tile([32, Ci], BF16, tag="xf")
                nc.vector.tensor_copy(xf[:], pxf[:])

                # ---- transpose xf -> xfT (Ci, 32); swapped/negated copy ----
                xfT = tmp1.tile([128, kt * 32], BF16, tag="xfT")
                for k in range(kt):
                    pt = pspec.tile([128, 32], BF16, tag="ptr")
                    w = min(128, Ci - k * 128)
                    nc.tensor.transpose(pt[:w, :], xf[:, k * 128:k * 128 + w],
                                        ident[:32, :32])
                    nc.vector.tensor_copy(xfT[:w, k * 32:(k + 1) * 32],
                                          pt[:w, :])
                xfN = tmp1.tile([128, kt * 32], BF16, tag="xfN")
                xfT3 = xfT[:].rearrange("p (a b) -> p a b", b=2)
                xfN3 = xfN[:].rearrange("p (a b) -> p a b", b=2)
                for k in range(kt):
                    mlo = k * 16
                    nc.scalar.mul(xfN3[:, mlo:mlo + 16, 0:1],
                                  xfT3[:, mlo:mlo + 16, 1:2], -1.0)
                    nc.vector.tensor_copy(xfN3[:, mlo:mlo + 16, 1:2],
                                          xfT3[:, mlo:mlo + 16, 0:1])

                # ---- mix (per mode) -> of (Co, 32) ----
                pof = pspec.tile([Co, 32], F32, tag="pof")
                for mu in range(16):
                    n_grp = 2 * kt
                    gi = 0
                    for k in range(kt):
                        w = min(128, Ci - k * 128)
                        mws = stm3.tile([128, 2 * Co], BF16, tag="mixw")
                        nc.sync.dma_start(
                            mws[:w].rearrange("i (j o) -> i j o", j=2),
                            mixw[tag][mu, :, k * 128:k * 128 + w]
                            .rearrange("j i o -> i j o"))
                        nc.tensor.matmul(
                            pof[:, 2 * mu:2 * mu + 2], mws[:w, :Co],
                            xfT[:w, k * 32 + 2 * mu:k * 32 + 2 * mu + 2],
                            start=(gi == 0), stop=(gi == n_grp - 1))
                        gi += 1
                        nc.tensor.matmul(
                            pof[:, 2 * mu:2 * mu + 2], mws[:w, Co:2 * Co],
                            xfN[:w, k * 32 + 2 * mu:k * 32 + 2 * mu + 2],
                            start=(gi == 0), stop=(gi == n_grp - 1))
                        gi += 1
                of_sb = tmp1.tile([Co, 32], BF16, tag="of_sb")
                nc.vector.tensor_copy(of_sb[:], pof[:])

                # ---- transpose of -> (32, Co); batch-select; AllReduce ----
                pofT = pspec.tile([32, 128], BF16, tag="ptr")
                nc.tensor.transpose(pofT[:, :Co], of_sb[:], ident[:Co, :Co])
                s0 = tmp1.tile([32, Co], F32, tag="sa", name="s0")
                s1 = tmp1.tile([32, Co], F32, tag="sb", name="s1")
                nc.scalar.activation(s0[:], pofT[:, :Co], AF.Copy,
                                     scale=bsel_t[:, 0:1])
                nc.scalar.activation(s1[:], pofT[:, :Co], AF.Copy,
                                     scale=bsel_t[:, 1:2])
                nc.sync.dma_start(cc_in[tag][0], s0[:])
                nc.sync.dma_start(cc_in[tag][1], s1[:])
                nc.gpsimd.collective_compute(
                    "AllReduce", mybir.AluOpType.add, replica_groups=ALL8,
                    ins=[cc_in[tag][:]], outs=[cc_out[tag][:]])
                t0 = tmp1.tile([32, Co], F32, tag="sa", name="t0")
                t1 = tmp1.tile([32, Co], F32, tag="sb", name="t1")
                nc.sync.dma_start(t0[:], cc_out[tag][0])
                nc.sync.dma_start(t1[:], cc_out[tag][1])
                u0 = tmp1.tile([32, Co], F32, tag="u0")
                nc.scalar.activation(u0[:], t0[:], AF.Copy,
                                     scale=bsel_t[:, 0:1])
                u1 = tmp1.tile([32, Co], F32, tag="u1")
                nc.scalar.activation(u1[:], t1[:], AF.Copy,
                                     scale=bsel_t[:, 1:2])
                nc.vector.tensor_add(u0[:], u0[:], u1[:])
                ofb = tmp1.tile([32, Co], BF16, tag="ofb")
                nc.vector.tensor_copy(ofb[:], u0[:])

                # ---- inverse expansion + 1x1 conv + gelu + skip ----
                moff = EMAX - e
                dv = act_view(dst_t, Co, e)
                nr = 2                             # rows per chunk
                npix = nr * WW
                for c_i in range(_rows(e) // nr):
                    rd = nr * c_i
                    ps = ppool.tile([Co, npix], F32, tag="cpsum")
                    if tag == "f5":
                        xch = stm2.tile([128, kt * npix], F32R, tag="cwx", name="xch")
                        for k in range(kt):
                            nc.sync.dma_start(
                                xch[:, k * npix:(k + 1) * npix],
                                x5_sl[k * 128:(k + 1) * 128,
                                      rd * WW:(rd + nr) * WW])
                        for k in range(kt):
                            nc.tensor.matmul(
                                ps[:], bwT_t[tag][:, k * Co:(k + 1) * Co],
                                xch[:, k * npix:(k + 1) * npix],
                                start=(k == 0), stop=False)
                    else:
                        sv = act_view(src_t, Ci, src_e)
                        nc.tensor.matmul(
                            ps[:], bwT_t[tag][:Ci, :Co],
                            sv[:, rd:rd + nr, 1:WP - 1],
                            start=True, stop=False)
                    gch = stm2.tile([32, npix], BF16, tag="gbch")
                    nc.sync.dma_start(
                        gch[:], gb[:, (rd + moff) * WW:(rd + moff + nr) * WW])
                    nc.tensor.matmul(ps[:], ofb[:, :Co], gch[:],
                                     start=False, stop=True)
                    dslice = dv[:, rd:rd + nr, 1:WP - 1]
                    nc.scalar.activation(
                        dslice, ps[:].rearrange("c (a w) -> c a w", w=WW),
                        AF.Gelu, bias=bb_t[tag][:Co, 0:1])
                    sk = stm2.tile([Co, npix], F32, tag="skipch")
                    nc.sync.dma_start(sk[:],
                                      skips[tag][:Co, rd * WW:(rd + nr) * WW])
                    nc.vector.tensor_add(
                        dslice, dslice,
                        sk[:].rearrange("c (a w) -> c a w", w=WW))
                band_mask(dst_t, Co, e)
                if dst_stack > 1:
                    stack_copies(dst_t, Co, e, dst_stack)

            # ---------------- conv stage ----------------
            def conv_stage(tag, C, st, e, src_t, src_e, dst_t,
                           dst_stack=1):
                nt, kw = CWSHP[tag][1], CWSHP[tag][2]
                wsl = stm2.tile([128, nt * C], CWDT[tag], tag="cwx",
                                name="wsl")
                nc.sync.dma_start(
                    wsl[:kw].rearrange("i (t o) -> i t o", t=nt),
                    cw[tag][st].rearrange("t i o -> i t o"))
                svf = src_t[:].rearrange("c (r w) -> c r w", w=WP)
                dvv = act_view(dst_t, C, e)
                for c_i in range(_rows(e) // 2):
                    rd = 2 * c_i
                    ps = ppool.tile([C, 512], F32, tag="cpsum")
                    if tag == "c6":
                        for t9 in range(9):
                            dy, dx = t9 // 3 - 1, t9 % 3 - 1
                            nc.tensor.matmul(
                                ps[:], wsl[:kw, t9 * C:(t9 + 1) * C],
                                svf[:C, rd + 1 + dy:rd + 3 + dy,
                                    1 + dx:WP - 1 + dx],
                                start=(t9 == 0), stop=(t9 == 8))
                    elif tag == "c7":
                        for j in range(6):
                            dx = j % 3 - 1
                            r0_ = rd if j < 3 else rd + 1
                            nc.tensor.matmul(
                                ps[:], wsl[:kw, j * C:(j + 1) * C],
                                svf[:kw, r0_:r0_ + 2, 1 + dx:WP - 1 + dx],
                                start=(j == 0), stop=(j == 5))
                    else:  # c8, c9: 3-stack, 3 MMs
                        for j in range(3):
                            dx = j - 1
                            nc.tensor.matmul(
                                ps[:], wsl[:kw, j * C:(j + 1) * C],
                                svf[:kw, rd:rd + 2, 1 + dx:WP - 1 + dx],
                                start=(j == 0), stop=(j == 2))
                    nc.scalar.activation(
                        dvv[:, rd:rd + 2, 1:WP - 1],
                        ps[:].rearrange("c (a w) -> c a w", w=WW), AF.Relu,
                        bias=cb_t[tag][:C, st:st + 1])
                band_mask(dst_t, C, e)
                if dst_stack > 1:
                    stack_copies(dst_t, C, e, dst_stack)

            # ---------------- the network ----------------
            x5u = new_act("A", 128, 9)
            fno_block("f5", 256, 128, 9, None, None, x5u)
            x6a = new_act("B", 128, 8)
            conv_stage("c6", 128, 0, 8, x5u, 9, x6a)
            x6b = new_act("A", 128, 7)
            conv_stage("c6", 128, 1, 7, x6a, 8, x6b)
            x6 = new_act("B", 128, 6)
            conv_stage("c6", 128, 2, 6, x6b, 7, x6)
            x6u = new_act("A", 64, 6)
            fno_block("f6", 128, 64, 6, x6, 6, x6u, dst_stack=2)
            x7a = new_act("B", 64, 5)
            conv_stage("c7", 64, 0, 5, x6u, 6, x7a, dst_stack=2)
            x7b = new_act("A", 64, 4)
            conv_stage("c7", 64, 1, 4, x7a, 5, x7b, dst_stack=2)
            x7 = new_act("B", 64, 3)
            conv_stage("c7", 64, 2, 3, x7b, 4, x7)
            x7u = new_act("A", 32, 3)
            fno_block("f7", 64, 32, 3, x7, 3, x7u, dst_stack=3)
            x8a = new_act("B", 32, 2)
            conv_stage("c8", 32, 0, 2, x7u, 3, x8a, dst_stack=3)
            x8 = new_act("A", 32, 1)
            conv_stage("c8", 32, 1, 1, x8a, 2, x8)
            x8u = new_act("B", 16, 1)
            fno_block("f8", 32, 16, 1, x8, 1, x8u, dst_stack=3)
            x9 = new_act("A", 16, 0)
            conv_stage("c9", 16, 0, 0, x8u, 1, x9)

            # final 1x1 conv (16 -> 2), owned rows only
            x9v = act_view(x9, 16, 0)
            for c_i in range(OWN // 2):
                rd = 2 * c_i
                ps = ppool.tile([2, 512], F32, tag="cpsum")
                nc.tensor.matmul(ps[:], ow_t[:], x9v[:, rd:rd + 2, 1:WP - 1],
                                 start=True, stop=True)
                oc = stm2.tile([2, 512], F32, tag="skipch", name="outch")
                nc.scalar.activation(oc[:], ps[:], AF.Identity, bias=ob_t[:])
                nc.sync.dma_start(out_sl[:, rd * WW:(rd + 2) * WW], oc[:])

    nc.compile()
    return nc


# ---------------------------------------------------------------------------
# host side
# ---------------------------------------------------------------------------

def _slice_rows(x, lo, hi):
    """x: (C, 256, 256) -> (C, hi-lo, 256) zero-padded out of range."""
    C = x.shape[0]
    out = np.zeros((C, hi - lo, WW), np.float32)
    a, b2 = max(lo, 0), min(hi, HH)
    if b2 > a:
        out[:, a - lo:b2 - lo] = x[:, a:b2]
    return out


def _host_inputs(inputs):
    i = inputs
    maps = []
    kk, ll = np.meshgrid(np.arange(M), np.arange(M), indexing="ij")
    kf = kk.reshape(-1).astype(np.float64)   # mu = 4k + l
    lf = ll.reshape(-1).astype(np.float64)
    alpha32 = np.where(lf == 0, 1.0, 2.0).repeat(2)  # per 32-comp row

    def basis(rows_abs, wvals):  # -> (32, len(rows), len(w))
        th = 2 * np.pi * (kf[:, None, None] * rows_abs[None, :, None] / HH
                          + lf[:, None, None] * wvals[None, None, :] / WW)
        out = np.empty((32, len(rows_abs), len(wvals)), np.float32)
        out[0::2] = np.cos(th) / 256.0
        out[1::2] = -np.sin(th) / 256.0
        return out

    # weights (identical on all cores)
    cw_np = {}
    cb_np = {}
    for tag, C, nst, _ in CONVS:
        w = np.asarray(i[tag + "_w"], np.float32)     # (n, co, ci, 3, 3)
        wt = w.transpose(0, 3, 4, 2, 1)               # (n, dy, dx, ci, co)
        if tag == "c6":
            cw_np[tag] = np.ascontiguousarray(wt.reshape(nst, 9, C, C))
        elif tag == "c7":
            cwv = np.zeros((nst, 6, 128, C), np.float32)
            for dx in range(3):
                cwv[:, dx, :C] = wt[:, 0, dx]         # dy=-1 via h0
                cwv[:, dx, C:2 * C] = wt[:, 1, dx]    # dy=0 via h1
                cwv[:, 3 + dx, C:2 * C] = wt[:, 2, dx]  # dy=+1 via h1
            cw_np[tag] = cwv
        else:  # c8, c9: 3-stack
            cwv = np.zeros((nst, 3, 3 * C, C), np.float32)
            for dx in range(3):
                for dy in range(3):
                    cwv[:, dx, dy * C:(dy + 1) * C] = wt[:, dy, dx]
            cw_np[tag] = cwv
        cbv = np.zeros((128, nst), np.float32)
        cbv[:C] = np.asarray(i[tag + "_b"], np.float32).T
        cb_np[tag] = cbv
    fno_np = {}
    for tag, Ci, Co, e in FNOS:
        kt = (Ci + 127) // 128
        bw = np.asarray(i[tag + "_bw"], np.float32)[:, :, 0, 0]  # (oc, ic)
        full = np.ascontiguousarray(bw.T)                        # (ic, oc)
        bwT_ = np.zeros((128, kt * Co), np.float32)
        for k in range(kt):
            w = min(128, Ci - k * 128)
            bwT_[:w, k * Co:(k + 1) * Co] = full[k * 128:k * 128 + w]
        wr = np.asarray(i[tag + "_wr"], np.float32)   # (ic, oc, 4, 4)
        wi = np.asarray(i[tag + "_wi"], np.float32)
        mw = np.zeros((16, 2, Ci, Co), np.float32)
        for mu in range(16):
            k, l = mu // 4, mu % 4
            mw[mu, 0] = wr[:, :, k, l]
            mw[mu, 1] = wi[:, :, k, l]
        bbv = np.zeros((128, 1), np.float32)
        bbv[:Co, 0] = np.asarray(i[tag + "_bb"], np.float32)
        fno_np[tag] = (bwT_, bbv, mw.astype(ml_dtypes.bfloat16))
    owT_np = np.ascontiguousarray(
        np.asarray(i["ow"], np.float32)[:, :, 0, 0].T)           # (16, 2)
    ob_np = np.asarray(i["ob"], np.float32)[:, None]

    skips_full = {"f5": np.asarray(i["x4"], np.float32),
                  "f6": np.asarray(i["x3"], np.float32),
                  "f7": np.asarray(i["x2"], np.float32),
                  "f8": np.asarray(i["x1"], np.float32)}
    x5 = np.asarray(i["x5"], np.float32)
    wvals = np.arange(WW, dtype=np.float64)
    K_rows9 = _rows(9)

    for core in range(NCORES):
        b, q = divmod(core, 4)
        r0 = OWN * q
        m = {}
        m["x5_sl"] = _slice_rows(x5[b], r0 - 9, r0 + OWN + 9).reshape(256, -1)
        m["x4_sl"] = _slice_rows(skips_full["f5"][b], r0 - 9, r0 + OWN + 9).reshape(128, -1)
        m["x3_sl"] = _slice_rows(skips_full["f6"][b], r0 - 6, r0 + OWN + 6).reshape(64, -1)
        m["x2_sl"] = _slice_rows(skips_full["f7"][b], r0 - 3, r0 + OWN + 3).reshape(32, -1)
        m["x1_sl"] = _slice_rows(skips_full["f8"][b], r0 - 1, r0 + OWN + 1).reshape(16, -1)
        xo = x5[b][:, r0:r0 + OWN, :]                       # (256c, 64h, 256w)
        x5T_ = xo.transpose(1, 2, 0).reshape(OWN, 2, 128, 256).transpose(1, 0, 2, 3)
        m["x5T"] = np.ascontiguousarray(x5T_).astype(ml_dtypes.bfloat16)
        # fwd basis (w, (wt h m)) at abs rows r0+h, col wt*128+w
        fbb = basis(np.arange(r0, r0 + OWN, dtype=np.float64), wvals)  # (32,64,256)
        fbb = (fbb.transpose(2, 1, 0)                       # (w256, h, m)
               .reshape(2, 128, OWN, 32)                    # (wt, w, h, m)
               .transpose(1, 0, 2, 3)                       # (w, wt, h, m)
               .reshape(128, 2 * OWN * 32))
        m["fb"] = np.ascontiguousarray(fbb).astype(ml_dtypes.bfloat16)
        rows = np.arange(r0 - 9, r0 + OWN + 9, dtype=np.float64)
        gbb = basis(rows, wvals) * alpha32[:, None, None]
        gbb[:, (rows < 0) | (rows >= HH), :] = 0.0
        m["gb"] = gbb.reshape(32, -1).astype(ml_dtypes.bfloat16)
        mrow = ((rows >= 0) & (rows < HH)).astype(np.float32)
        m["mask_pp"] = np.tile(mrow[None, :], (128, 1))
        bs = np.zeros((32, 2), np.float32)
        bs[:, b] = 1.0
        m["bsel"] = bs
        m["zpad"] = np.zeros((128, 2 * K_rows9), np.float32)
        m["zpadb"] = np.zeros((128, 2 * K_rows9), ml_dtypes.bfloat16)
        for tag, C, nst, _ in CONVS:
            m[tag + "w"] = cw_np[tag]
            m[tag + "b"] = cb_np[tag]
        for tag, Ci, Co, e in FNOS:
            bwT_, bb_, mw_ = fno_np[tag]
            m[tag + "_bwT"] = bwT_
            m[tag + "_bb"] = bb_
            m[tag + "_mixw"] = mw_
        m["owT"] = owT_np
        m["ob"] = ob_np
        maps.append(m)
    return maps


_NC_CACHE = {}


def kernel(**inputs):
    if "nc" not in _NC_CACHE:
        _NC_CACHE["nc"] = _build_nc()
    nc = _NC_CACHE["nc"]
    maps = _host_inputs(inputs)
    res = run_bass_kernel_spmd(nc, maps, list(range(NCORES)), trace=False)
    out = np.zeros((B, 2, HH, WW), np.float32)
    for core in range(NCORES):
        b, q = divmod(core, 4)
        r0 = OWN * q
        out[b, :, r0:r0 + OWN, :] = res.results[core]["out_sl"].reshape(2, OWN, WW)
    return out
